# revision 1
# baseline (speedup 1.0000x reference)
"""AttentionBlock (GroupNorm -> qkv conv1x1 -> 4-head attention -> proj -> residual)
as a distributed Bass/Tile kernel on 8 TRN2 NeuronCores.

Sharding: core j handles batch b = j//2 and query-half h = j%2 (2048 of the 4096
spatial positions). K/V are computed full-length per core (cheap), so the proj
output slices are disjoint across cores and no collectives are needed.

Self-contained: hardcodes all shapes; host side only reshapes/shards inputs,
transposes/folds weights, and reassembles the 8 output slices.
"""
import numpy as np
import ml_dtypes

import concourse.bass as bass
import concourse.bacc as bacc
import concourse.tile as tile
from concourse import mybir
from concourse import bass_utils
from concourse.bass_interp import get_hw_module

F32 = mybir.dt.float32
BF16 = mybir.dt.bfloat16
BF = ml_dtypes.bfloat16

B, C, Himg, Wimg = 4, 256, 64, 64
T = Himg * Wimg            # 4096 tokens
HEADS, D = 4, 64           # 4 heads x 64 dims
GROUPS, GS = 32, 8         # groupnorm: 32 groups of 8 channels
EPS = 1e-5
TQ = T // 2                # queries per core (2048)
NTT = TQ // 512            # query tiles of 512
NSC = T // 128             # key chunks of 128
SCALE = 1.0 / np.sqrt(D)

_CACHED = {}


def _build():
    nc = bacc.Bacc("TRN2", target_bir_lowering=False, debug=False,
                   enable_asserts=False, num_devices=8)

    xb_d = nc.dram_tensor("xb", [C, T], F32, kind="ExternalInput")
    xq_d = nc.dram_tensor("xq", [C, TQ], F32, kind="ExternalInput")
    qkvT_d = nc.dram_tensor("qkvT", [C, 3 * C], BF16, kind="ExternalInput")
    qkvb_d = nc.dram_tensor("qkvb", [3 * C, 1], F32, kind="ExternalInput")
    projT_d = nc.dram_tensor("projT", [HEADS, D, C], BF16, kind="ExternalInput")
    projb_d = nc.dram_tensor("projb", [C, 1], F32, kind="ExternalInput")
    gmat_d = nc.dram_tensor("gmat", [128, 16], F32, kind="ExternalInput")
    gmatT_d = nc.dram_tensor("gmatT", [16, 128], F32, kind="ExternalInput")
    out_d = nc.dram_tensor("out", [C, TQ], F32, kind="ExternalOutput")

    with tile.TileContext(nc) as tc:
        with (
            tc.tile_pool(name="consts", bufs=1) as consts,
            tc.tile_pool(name="data", bufs=1) as data,
            tc.tile_pool(name="gn", bufs=1) as gn,
            tc.tile_pool(name="pt", bufs=3) as ppool,
            tc.tile_pool(name="dn", bufs=2) as dn,
            tc.tile_pool(name="ao", bufs=4) as ao,
            tc.tile_pool(name="ah", bufs=8) as ahpool,
            tc.tile_pool(name="dr", bufs=2, space="DRAM") as drpool,
            tc.tile_pool(name="ps", bufs=2, space="PSUM") as psum_s,
            tc.tile_pool(name="pa", bufs=2, space="PSUM") as psum_a,
        ):
            # ---------------- constant / weight loads ----------------
            qkvT_sb, qb_sb, kb_sb, pb_sb = [], [], [], []
            for ct in range(2):
                w = consts.tile([128, 3 * C], BF16, tag=f"qkvT{ct}", name=f"qkvT{ct}")
                nc.sync.dma_start(w[:], qkvT_d.ap()[ct * 128:(ct + 1) * 128, :])
                qkvT_sb.append(w)
                qb = consts.tile([128, 1], F32, tag=f"qb{ct}", name=f"qb{ct}")
                nc.sync.dma_start(qb[:], qkvb_d.ap()[ct * 128:(ct + 1) * 128, :])
                qb_sb.append(qb)
                kb = consts.tile([128, 1], F32, tag=f"kb{ct}", name=f"kb{ct}")
                nc.sync.dma_start(kb[:], qkvb_d.ap()[C + ct * 128:C + (ct + 1) * 128, :])
                kb_sb.append(kb)
                pb = consts.tile([128, 1], F32, tag=f"pb{ct}", name=f"pb{ct}")
                nc.sync.dma_start(pb[:], projb_d.ap()[ct * 128:(ct + 1) * 128, :])
                pb_sb.append(pb)
            projT_sb = []
            for h in range(HEADS):
                pw = consts.tile([D, C], BF16, tag=f"projT{h}", name=f"projT{h}")
                nc.sync.dma_start(pw[:], projT_d.ap()[h])
                projT_sb.append(pw)
            gmat_sb = consts.tile([128, 16], F32, tag="gmat", name="gmat")
            nc.sync.dma_start(gmat_sb[:], gmat_d.ap()[:])
            gmatT_sb = consts.tile([16, 128], F32, tag="gmatT", name="gmatT")
            nc.sync.dma_start(gmatT_sb[:], gmatT_d.ap()[:])
            # v-bias broadcast along partitions: [128, 256] from qkvb[512:768]
            bvT_sb = consts.tile([128, C], F32, tag="bvT", name="bvT")
            bvT_src = bass.AP(tensor=qkvb_d, offset=2 * C, ap=[[0, 128], [1, C]])
            nc.sync.dma_start(bvT_sb[:], bvT_src)
            eps_t = gn.tile([16, 1], F32, tag="eps", name="eps")
            nc.vector.memset(eps_t[:], EPS)

            # ---------------- x loads ----------------
            xb_sb, xq_sb = [], []
            for ct in range(2):
                xt = data.tile([128, T], F32, tag=f"xb{ct}", name=f"xb{ct}")
                nc.sync.dma_start(xt[:], xb_d.ap()[ct * 128:(ct + 1) * 128, :])
                xb_sb.append(xt)
                xqt = data.tile([128, TQ], F32, tag=f"xq{ct}", name=f"xq{ct}")
                nc.sync.dma_start(xqt[:], xq_d.ap()[ct * 128:(ct + 1) * 128, :])
                xq_sb.append(xqt)

            # ---------------- GroupNorm statistics ----------------
            # per-channel mean/var via bn_stats, then 8-channel group
            # aggregation via tiny PE matmuls with the group matrices.
            stats2 = []
            for ct in range(2):
                st = gn.tile([128, 8, 6], F32, tag=f"st{ct}", name=f"st{ct}")
                for i in range(8):
                    nc.vector.bn_stats(st[:, i, :], xb_sb[ct][:, i * 512:(i + 1) * 512])
                mv = gn.tile([128, 2], F32, tag=f"mv{ct}", name=f"mv{ct}")
                nc.vector.bn_aggr(mv[:], st[:])
                s2 = gn.tile([128, 2], F32, tag=f"s2{ct}", name=f"s2{ct}")
                nc.vector.tensor_copy(s2[:, 0:1], mv[:, 0:1])
                m2 = gn.tile([128, 1], F32, tag=f"m2{ct}", name=f"m2{ct}")
                nc.vector.tensor_mul(m2[:], mv[:, 0:1], mv[:, 0:1])
                nc.vector.tensor_add(s2[:, 1:2], m2[:], mv[:, 1:2])
                stats2.append(s2)

            # group (mean, E[x^2]) per channel tile -> [16, 2] each
            gs_ps, vg_l, bc_sb = [], [], []
            vg = gn.tile([16, 2], F32, tag="vg", name="vg")
            for ct in range(2):
                g1 = psum_s.tile([16, 2], F32, tag="ps", name="ps")
                nc.tensor.matmul(g1[:], gmat_sb[:], stats2[ct][:],
                                 start=True, stop=True)
                gsb = gn.tile([16, 2], F32, tag=f"gsb{ct}", name=f"gsb{ct}")
                nc.vector.tensor_copy(gsb[:], g1[:])
                gs_ps.append(gsb)
                m2g = gn.tile([16, 1], F32, tag=f"m2g{ct}", name=f"m2g{ct}")
                nc.vector.tensor_mul(m2g[:], gsb[:, 0:1], gsb[:, 0:1])
                nc.vector.tensor_sub(vg[:, ct:ct + 1], gsb[:, 1:2], m2g[:])
            sg = gn.tile([16, 2], F32, tag="sg", name="sg")
            nc.scalar.activation(sg[:], vg[:], mybir.ActivationFunctionType.Sqrt,
                                 bias=eps_t[:])
            rg = gn.tile([16, 2], F32, tag="rg", name="rg")
            nc.vector.reciprocal(rg[:], sg[:])
            for ct in range(2):
                bcv = gn.tile([16, 2], F32, tag=f"bcv{ct}", name=f"bcv{ct}")
                nc.vector.tensor_copy(bcv[:, 0:1], gs_ps[ct][:, 0:1])
                nc.vector.tensor_copy(bcv[:, 1:2], rg[:, ct:ct + 1])
                b1 = psum_s.tile([128, 2], F32, tag="ps", name="ps")
                nc.tensor.matmul(b1[:], gmatT_sb[:], bcv[:],
                                 start=True, stop=True)
                bsb = gn.tile([128, 2], F32, tag=f"bc{ct}", name=f"bc{ct}")
                nc.vector.tensor_copy(bsb[:], b1[:])
                bc_sb.append(bsb)

            # ---------------- apply GN -> xn (bf16) ----------------
            xn_sb, xnq_sb = [], []
            for ct in range(2):
                xn = data.tile([128, T], BF16, tag=f"xn{ct}", name=f"xn{ct}")
                nc.vector.tensor_scalar(
                    out=xn[:], in0=xb_sb[ct][:],
                    scalar1=bc_sb[ct][:, 0:1], scalar2=bc_sb[ct][:, 1:2],
                    op0=mybir.AluOpType.subtract, op1=mybir.AluOpType.mult)
                xn_sb.append(xn)
                xnq = data.tile([128, TQ], BF16, tag=f"xnq{ct}", name=f"xnq{ct}")
                nc.vector.tensor_scalar(
                    out=xnq[:], in0=xq_sb[ct][:],
                    scalar1=bc_sb[ct][:, 0:1], scalar2=bc_sb[ct][:, 1:2],
                    op0=mybir.AluOpType.subtract, op1=mybir.AluOpType.mult)
                xnq_sb.append(xnq)

            # ---------------- K (full length), Q (this half) ----------------
            k_sb = [data.tile([128, T], BF16, tag=f"k{p}", name=f"k{p}")
                    for p in range(2)]
            for p in range(2):
                for t8 in range(8):
                    kv_ps = psum_s.tile([128, 512], F32, tag="ps", name="ps")
                    for ct in range(2):
                        nc.tensor.matmul(
                            kv_ps[:],
                            qkvT_sb[ct][:, C + p * 128:C + (p + 1) * 128],
                            xn_sb[ct][:, t8 * 512:(t8 + 1) * 512],
                            start=(ct == 0), stop=(ct == 1))
                    nc.vector.tensor_scalar_add(
                        out=k_sb[p][:, t8 * 512:(t8 + 1) * 512],
                        in0=kv_ps[:], scalar1=kb_sb[p][:])

            q_sb = [data.tile([128, TQ], BF16, tag=f"q{p}", name=f"q{p}")
                    for p in range(2)]
            for p in range(2):
                for t4 in range(NTT):
                    q_ps = psum_s.tile([128, 512], F32, tag="ps", name="ps")
                    for ct in range(2):
                        nc.tensor.matmul(
                            q_ps[:],
                            qkvT_sb[ct][:, p * 128:(p + 1) * 128],
                            xnq_sb[ct][:, t4 * 512:(t4 + 1) * 512],
                            start=(ct == 0), stop=(ct == 1))
                    nc.vector.tensor_scalar_add(
                        out=q_sb[p][:, t4 * 512:(t4 + 1) * 512],
                        in0=q_ps[:], scalar1=qb_sb[p][:])

            # ---------------- vT: [s, head*65] with ones column per head ------
            # vt[:, i*260 + h*65 + j] = V[h*64+j, i*128:...]^T ; col h*65+64 == 1
            vt_sb = data.tile([128, NSC * 260], BF16, tag="vt", name="vt")
            ones_cols = vt_sb[:].rearrange("p (s h c) -> p s h c", s=NSC, c=65)
            nc.vector.memset(ones_cols[:, :, :, 64:65], 1.0)
            for i in range(NSC):
                vt_ps = psum_s.tile([128, C], F32, tag="ps", name="ps")
                for ct in range(2):
                    nc.tensor.matmul(
                        vt_ps[:],
                        xn_sb[ct][:, i * 128:(i + 1) * 128],
                        qkvT_sb[ct][:, 2 * C:3 * C],
                        start=(ct == 0), stop=(ct == 1))
                dst = vt_sb[:, i * 260:(i + 1) * 260].rearrange(
                    "p (h c) -> p h c", c=65)[:, :, 0:64]
                nc.vector.tensor_add(
                    dst,
                    vt_ps[:].rearrange("p (h c) -> p h c", c=64),
                    bvT_sb[:].rearrange("p (h c) -> p h c", c=64))

            # ---------------- attention + proj ----------------
            for tt in range(NTT):
                a_heads = [None] * HEADS
                for p in range(2):
                    a_ps = psum_a.tile([65, 1024], F32, tag="pa", name="pa")

                    def s_matmuls(i, p=p):
                        s_ps = psum_s.tile([128, 1024], F32, tag="ps", name="ps")
                        for u in range(2):
                            nc.tensor.matmul(
                                s_ps[:, u * 512:(u + 1) * 512],
                                k_sb[p][u * 64:(u + 1) * 64, i * 128:(i + 1) * 128],
                                q_sb[p][u * 64:(u + 1) * 64, tt * 512:(tt + 1) * 512],
                                start=True, stop=True,
                                tile_position=(u * 64, 0))
                        return s_ps

                    s_cur = s_matmuls(0)
                    for i in range(NSC):
                        p_t = ppool.tile([128, 1024], BF16, tag="pt", name="pt")
                        nc.scalar.activation(p_t[:], s_cur[:],
                                             mybir.ActivationFunctionType.Exp,
                                             scale=float(SCALE))
                        if i + 1 < NSC:
                            s_cur = s_matmuls(i + 1)
                        for u in range(2):
                            h = 2 * p + u
                            nc.tensor.matmul(
                                a_ps[:, u * 512:(u + 1) * 512],
                                vt_sb[:, i * 260 + h * 65:i * 260 + h * 65 + 65],
                                p_t[:, u * 512:(u + 1) * 512],
                                start=(i == 0), stop=(i == NSC - 1))

                    # reciprocal of the accumulated denominators (row 64)
                    d_inv = dn.tile([65, 1024], F32, tag="dinv", name="dinv")
                    nc.vector.reciprocal(d_inv[64:65, :], a_ps[64:65, :])
                    # broadcast along partitions via a DRAM bounce (SBUF
                    # sources must have nonzero partition stride)
                    d_dram = drpool.tile([1, 1024], F32, tag="ddr", name="ddr")
                    nc.sync.dma_start(d_dram[:], d_inv[64:65, :])
                    d_bc = dn.tile([64, 1024], F32, tag="dbc", name="dbc")
                    nc.sync.dma_start(d_bc[:], d_dram[:].to_broadcast([64, 1024]))
                    for u in range(2):
                        h = 2 * p + u
                        ah = ahpool.tile([D, 512], BF16, tag="ah", name="ah")
                        nc.vector.tensor_mul(
                            ah[:],
                            a_ps[0:64, u * 512:(u + 1) * 512],
                            d_bc[:, u * 512:(u + 1) * 512])
                        a_heads[h] = ah

                for oc in range(2):
                    pr_ps = psum_s.tile([128, 512], F32, tag="ps", name="ps")
                    for h in range(HEADS):
                        nc.tensor.matmul(
                            pr_ps[:],
                            projT_sb[h][:, oc * 128:(oc + 1) * 128],
                            a_heads[h][:],
                            start=(h == 0), stop=(h == HEADS - 1))
                    o1 = ao.tile([128, 512], F32, tag="o1", name="o1")
                    nc.vector.tensor_scalar_add(out=o1[:], in0=pr_ps[:],
                                                scalar1=pb_sb[oc][:])
                    o2 = ao.tile([128, 512], F32, tag="o2", name="o2")
                    nc.vector.tensor_add(o2[:], o1[:],
                                         xq_sb[oc][:, tt * 512:(tt + 1) * 512])
                    nc.sync.dma_start(
                        out_d.ap()[oc * 128:(oc + 1) * 128, tt * 512:(tt + 1) * 512],
                        o2[:])

    nc.compile()
    nc.m = get_hw_module(nc.m)
    return nc


def _host_prep(inputs):
    x = np.asarray(inputs["x"], np.float32)
    gn_w = np.asarray(inputs["gn_weight"], np.float32)
    gn_b = np.asarray(inputs["gn_bias"], np.float32)
    qkv_w = np.asarray(inputs["qkv_w"], np.float32)
    qkv_b = np.asarray(inputs["qkv_b"], np.float32)
    proj_w = np.asarray(inputs["proj_w"], np.float32)
    proj_b = np.asarray(inputs["proj_b"], np.float32)

    W_ = qkv_w * gn_w[None, :]
    b_ = qkv_w @ gn_b + qkv_b
    qkvT = np.ascontiguousarray(W_.T).astype(BF)
    # proj_w.T is [c_in(=head*d), c_out]; split head dim so each head's
    # 64 rows sit on partitions 0..63
    projT = np.ascontiguousarray(proj_w.T.reshape(HEADS, D, C)).astype(BF)

    gmat = np.zeros((128, 16), np.float32)
    gmatT = np.zeros((16, 128), np.float32)
    for ch in range(128):
        gmat[ch, ch // GS] = 1.0 / GS
        gmatT[ch // GS, ch] = 1.0
    shared = {
        "qkvT": qkvT,
        "qkvb": b_.reshape(3 * C, 1).astype(np.float32),
        "projT": projT,
        "projb": proj_b.reshape(C, 1).astype(np.float32),
        "gmat": gmat,
        "gmatT": gmatT,
    }
    x3 = x.reshape(B, C, T)
    in_maps = []
    for j in range(8):
        b, hf = j // 2, j % 2
        m = dict(shared)
        m["xb"] = np.ascontiguousarray(x3[b])
        m["xq"] = np.ascontiguousarray(x3[b][:, hf * TQ:(hf + 1) * TQ])
        in_maps.append(m)
    return x, in_maps


def kernel(**inputs) -> np.ndarray:
    if "nc" not in _CACHED:
        _CACHED["nc"] = _build()
    nc = _CACHED["nc"]
    x, in_maps = _host_prep(inputs)
    res = bass_utils.run_bass_kernel_spmd(nc, in_maps, core_ids=list(range(8)))
    out = np.zeros((B, C, T), np.float32)
    for j in range(8):
        b, hf = j // 2, j % 2
        out[b][:, hf * TQ:(hf + 1) * TQ] = np.asarray(
            res.results[j]["out"], np.float32)
    return out.reshape(B, C, Himg, Wimg)



# revision 2
# speedup vs baseline: 1.0347x; 1.0347x over previous
"""AttentionBlock (GroupNorm -> qkv conv1x1 -> 4-head attention -> proj -> residual)
as a distributed Bass/Tile kernel on 8 TRN2 NeuronCores.

Sharding: core j handles batch b = j//2 and query-half h = j%2 (2048 of the 4096
spatial positions). K/V are computed full-length per core (cheap), so the proj
output slices are disjoint across cores and no collectives are needed.

Engine split: exp of the attention scores is the hard bottleneck (only the Act
engine has exp, 128 lanes @ 1.2 GHz). So ~2/3 of the score chunks use a
Schraudolph-style exp approximation on the Vector engine (one tensor_scalar
producing int16 bits that reinterpret as bf16), the rest use exact exp on Act.
Denominator broadcast + normalize run on GPSIMD; GN apply runs on Act.

Self-contained: hardcodes all shapes; host side only reshapes/shards inputs,
transposes/folds weights, and reassembles the 8 output slices.
"""
import numpy as np
import ml_dtypes

import concourse.bass as bass
import concourse.bacc as bacc
import concourse.tile as tile
from concourse import mybir
from concourse import bass_utils
from concourse.bass_interp import get_hw_module

F32 = mybir.dt.float32
BF16 = mybir.dt.bfloat16
I16 = mybir.dt.int16
BF = ml_dtypes.bfloat16

B, C, Himg, Wimg = 4, 256, 64, 64
T = Himg * Wimg            # 4096 tokens
HEADS, D = 4, 64           # 4 heads x 64 dims
GROUPS, GS = 32, 8         # groupnorm: 32 groups of 8 channels
EPS = 1e-5
TQ = T // 2                # queries per core (2048)
NTT = TQ // 512            # query tiles of 512
NSC = T // 128             # key chunks of 128
SCALE = 1.0 / np.sqrt(D)

# Schraudolph exp-approx constants (bf16 bit trick):
#   bits16 = round(ALPHA * s + BETA);  bits16 viewed as bf16 ~ exp(SCALE * s)
# BETA tuned so the approximation is unbiased (E[approx/exp] = 1), which makes
# mixing approx (DVE) and exact (Act) chunks inside one softmax safe.
ALPHA = float(128.0 * np.log2(np.e) * SCALE)
BETA = float(127.0 * 128.0 - 7.5)
# chunks of each (tt, p) iteration assigned to exact exp on the Act engine;
# the rest use the DVE approximation. Spread out, none adjacent, none at the
# end (so Act is free for the a_ps -> SBUF eviction copy).
ACT_CHUNKS = frozenset({0, 3, 6, 8, 11, 14, 16, 19, 22, 24, 27})

_CACHED = {}


def _build():
    nc = bacc.Bacc("TRN2", target_bir_lowering=False, debug=False,
                   enable_asserts=False, num_devices=8)

    xb_d = nc.dram_tensor("xb", [C, T], F32, kind="ExternalInput")
    xq_d = nc.dram_tensor("xq", [C, TQ], F32, kind="ExternalInput")
    qkvT_d = nc.dram_tensor("qkvT", [C, 3 * C], BF16, kind="ExternalInput")
    qkvb_d = nc.dram_tensor("qkvb", [3 * C, 1], F32, kind="ExternalInput")
    projT_d = nc.dram_tensor("projT", [HEADS, D, C], BF16, kind="ExternalInput")
    projb_d = nc.dram_tensor("projb", [C, 1], F32, kind="ExternalInput")
    gmat_d = nc.dram_tensor("gmat", [128, 16], F32, kind="ExternalInput")
    gmatT_d = nc.dram_tensor("gmatT", [16, 128], F32, kind="ExternalInput")
    out_d = nc.dram_tensor("out", [C, TQ], F32, kind="ExternalOutput")

    with tile.TileContext(nc) as tc:
        with (
            tc.tile_pool(name="consts", bufs=1) as consts,
            tc.tile_pool(name="data", bufs=1) as data,
            tc.tile_pool(name="gn", bufs=1) as gn,
            tc.tile_pool(name="pt", bufs=4) as ppool,
            tc.tile_pool(name="dn", bufs=2) as dn,
            tc.tile_pool(name="af", bufs=2) as afpool,
            tc.tile_pool(name="ao", bufs=4) as ao,
            tc.tile_pool(name="ah", bufs=4) as ahpool,
            tc.tile_pool(name="ps", bufs=3, space="PSUM") as psum_s,
            tc.tile_pool(name="pa", bufs=1, space="PSUM") as psum_a,
        ):
            # ---------------- constant / weight loads ----------------
            qkvT_sb, qb_sb, kb_sb, pb_sb = [], [], [], []
            for ct in range(2):
                w = consts.tile([128, 3 * C], BF16, tag=f"qkvT{ct}", name=f"qkvT{ct}")
                nc.sync.dma_start(w[:], qkvT_d.ap()[ct * 128:(ct + 1) * 128, :])
                qkvT_sb.append(w)
                qb = consts.tile([128, 1], F32, tag=f"qb{ct}", name=f"qb{ct}")
                nc.sync.dma_start(qb[:], qkvb_d.ap()[ct * 128:(ct + 1) * 128, :])
                qb_sb.append(qb)
                kb = consts.tile([128, 1], F32, tag=f"kb{ct}", name=f"kb{ct}")
                nc.sync.dma_start(kb[:], qkvb_d.ap()[C + ct * 128:C + (ct + 1) * 128, :])
                kb_sb.append(kb)
                pb = consts.tile([128, 1], F32, tag=f"pb{ct}", name=f"pb{ct}")
                nc.sync.dma_start(pb[:], projb_d.ap()[ct * 128:(ct + 1) * 128, :])
                pb_sb.append(pb)
            projT_sb = []
            for h in range(HEADS):
                pw = consts.tile([D, C], BF16, tag=f"projT{h}", name=f"projT{h}")
                nc.sync.dma_start(pw[:], projT_d.ap()[h])
                projT_sb.append(pw)
            gmat_sb = consts.tile([128, 16], F32, tag="gmat", name="gmat")
            nc.sync.dma_start(gmat_sb[:], gmat_d.ap()[:])
            gmatT_sb = consts.tile([16, 128], F32, tag="gmatT", name="gmatT")
            nc.sync.dma_start(gmatT_sb[:], gmatT_d.ap()[:])
            # v-bias broadcast along partitions: [128, 256] from qkvb[512:768]
            bvT_sb = consts.tile([128, C], F32, tag="bvT", name="bvT")
            bvT_src = bass.AP(tensor=qkvb_d, offset=2 * C, ap=[[0, 128], [1, C]])
            nc.sync.dma_start(bvT_sb[:], bvT_src)
            eps_t = gn.tile([16, 1], F32, tag="eps", name="eps")
            nc.vector.memset(eps_t[:], EPS)

            # ---------------- x loads (column-split so stats start early) ----
            xb_sb, xq_sb = [], []
            for ct in range(2):
                xt = data.tile([128, T], F32, tag=f"xb{ct}", name=f"xb{ct}")
                for t8 in range(8):
                    nc.sync.dma_start(
                        xt[:, t8 * 512:(t8 + 1) * 512],
                        xb_d.ap()[ct * 128:(ct + 1) * 128, t8 * 512:(t8 + 1) * 512])
                xb_sb.append(xt)
                xqt = data.tile([128, TQ], F32, tag=f"xq{ct}", name=f"xq{ct}")
                for t4 in range(4):
                    nc.sync.dma_start(
                        xqt[:, t4 * 512:(t4 + 1) * 512],
                        xq_d.ap()[ct * 128:(ct + 1) * 128, t4 * 512:(t4 + 1) * 512])
                xq_sb.append(xqt)

            # ---------------- GroupNorm statistics ----------------
            # per-channel mean/var via bn_stats, then 8-channel group
            # aggregation via tiny PE matmuls with the group matrices.
            stats2 = []
            for ct in range(2):
                st = gn.tile([128, 8, 6], F32, tag=f"st{ct}", name=f"st{ct}")
                for i in range(8):
                    nc.vector.bn_stats(st[:, i, :], xb_sb[ct][:, i * 512:(i + 1) * 512])
                mv = gn.tile([128, 2], F32, tag=f"mv{ct}", name=f"mv{ct}")
                nc.vector.bn_aggr(mv[:], st[:])
                s2 = gn.tile([128, 2], F32, tag=f"s2{ct}", name=f"s2{ct}")
                nc.vector.tensor_copy(s2[:, 0:1], mv[:, 0:1])
                m2 = gn.tile([128, 1], F32, tag=f"m2{ct}", name=f"m2{ct}")
                nc.vector.tensor_mul(m2[:], mv[:, 0:1], mv[:, 0:1])
                nc.vector.tensor_add(s2[:, 1:2], m2[:], mv[:, 1:2])
                stats2.append(s2)

            # group (mean, E[x^2]) per channel tile -> [16, 2] each
            gs_ps, bc_sb = [], []
            vg = gn.tile([16, 2], F32, tag="vg", name="vg")
            for ct in range(2):
                g1 = psum_s.tile([16, 2], F32, tag="ps", name="ps")
                nc.tensor.matmul(g1[:], gmat_sb[:], stats2[ct][:],
                                 start=True, stop=True)
                gsb = gn.tile([16, 2], F32, tag=f"gsb{ct}", name=f"gsb{ct}")
                nc.vector.tensor_copy(gsb[:], g1[:])
                gs_ps.append(gsb)
                m2g = gn.tile([16, 1], F32, tag=f"m2g{ct}", name=f"m2g{ct}")
                nc.vector.tensor_mul(m2g[:], gsb[:, 0:1], gsb[:, 0:1])
                nc.vector.tensor_sub(vg[:, ct:ct + 1], gsb[:, 1:2], m2g[:])
            sg = gn.tile([16, 2], F32, tag="sg", name="sg")
            nc.scalar.activation(sg[:], vg[:], mybir.ActivationFunctionType.Sqrt,
                                 bias=eps_t[:])
            rg = gn.tile([16, 2], F32, tag="rg", name="rg")
            nc.vector.reciprocal(rg[:], sg[:])
            negmr_sb = []
            for ct in range(2):
                bcv = gn.tile([16, 2], F32, tag=f"bcv{ct}", name=f"bcv{ct}")
                nc.vector.tensor_copy(bcv[:, 0:1], gs_ps[ct][:, 0:1])
                nc.vector.tensor_copy(bcv[:, 1:2], rg[:, ct:ct + 1])
                b1 = psum_s.tile([128, 2], F32, tag="ps", name="ps")
                nc.tensor.matmul(b1[:], gmatT_sb[:], bcv[:],
                                 start=True, stop=True)
                bsb = gn.tile([128, 2], F32, tag=f"bc{ct}", name=f"bc{ct}")
                nc.vector.tensor_copy(bsb[:], b1[:])
                bc_sb.append(bsb)
                # -mean * rstd, for the fused (x*r + b) GN apply on Act
                nmr = gn.tile([128, 1], F32, tag=f"nmr{ct}", name=f"nmr{ct}")
                nc.vector.scalar_tensor_tensor(
                    out=nmr[:], in0=bsb[:, 0:1], scalar=-1.0, in1=bsb[:, 1:2],
                    op0=mybir.AluOpType.mult, op1=mybir.AluOpType.mult)
                negmr_sb.append(nmr)

            # ---------------- apply GN -> xn (bf16), on the Act engine ------
            xn_sb, xnq_sb = [], []
            for ct in range(2):
                xn = data.tile([128, T], BF16, tag=f"xn{ct}", name=f"xn{ct}")
                for t8 in range(2):
                    nc.scalar.activation(
                        xn[:, t8 * 2048:(t8 + 1) * 2048],
                        xb_sb[ct][:, t8 * 2048:(t8 + 1) * 2048],
                        mybir.ActivationFunctionType.Identity,
                        scale=bc_sb[ct][:, 1:2], bias=negmr_sb[ct][:])
                xn_sb.append(xn)
                xnq = data.tile([128, TQ], BF16, tag=f"xnq{ct}", name=f"xnq{ct}")
                nc.scalar.activation(
                    xnq[:], xq_sb[ct][:],
                    mybir.ActivationFunctionType.Identity,
                    scale=bc_sb[ct][:, 1:2], bias=negmr_sb[ct][:])
                xnq_sb.append(xnq)

            # ---------------- K (full length), Q (this half) ----------------
            k_sb = [data.tile([128, T], BF16, tag=f"k{p}", name=f"k{p}")
                    for p in range(2)]
            for p in range(2):
                for t8 in range(8):
                    kv_ps = psum_s.tile([128, 512], F32, tag="ps", name="ps")
                    for ct in range(2):
                        nc.tensor.matmul(
                            kv_ps[:],
                            qkvT_sb[ct][:, C + p * 128:C + (p + 1) * 128],
                            xn_sb[ct][:, t8 * 512:(t8 + 1) * 512],
                            start=(ct == 0), stop=(ct == 1))
                    nc.vector.tensor_scalar_add(
                        out=k_sb[p][:, t8 * 512:(t8 + 1) * 512],
                        in0=kv_ps[:], scalar1=kb_sb[p][:])

            q_sb = [data.tile([128, TQ], BF16, tag=f"q{p}", name=f"q{p}")
                    for p in range(2)]
            for p in range(2):
                for t4 in range(NTT):
                    q_ps = psum_s.tile([128, 512], F32, tag="ps", name="ps")
                    for ct in range(2):
                        nc.tensor.matmul(
                            q_ps[:],
                            qkvT_sb[ct][:, p * 128:(p + 1) * 128],
                            xnq_sb[ct][:, t4 * 512:(t4 + 1) * 512],
                            start=(ct == 0), stop=(ct == 1))
                    nc.vector.tensor_scalar_add(
                        out=q_sb[p][:, t4 * 512:(t4 + 1) * 512],
                        in0=q_ps[:], scalar1=qb_sb[p][:])

            # ---------------- vT: [s, head*65] with ones column per head ------
            # vt[:, i*260 + h*65 + j] = V[h*64+j, i*128:...]^T ; col h*65+64 == 1
            vt_sb = data.tile([128, NSC * 260], BF16, tag="vt", name="vt")
            ones_cols = vt_sb[:].rearrange("p (s h c) -> p s h c", s=NSC, c=65)
            nc.vector.memset(ones_cols[:, :, :, 64:65], 1.0)
            for i in range(NSC):
                vt_ps = psum_s.tile([128, C], F32, tag="ps", name="ps")
                for ct in range(2):
                    nc.tensor.matmul(
                        vt_ps[:],
                        xn_sb[ct][:, i * 128:(i + 1) * 128],
                        qkvT_sb[ct][:, 2 * C:3 * C],
                        start=(ct == 0), stop=(ct == 1))
                dst = vt_sb[:, i * 260:(i + 1) * 260].rearrange(
                    "p (h c) -> p h c", c=65)[:, :, 0:64]
                nc.vector.tensor_add(
                    dst,
                    vt_ps[:].rearrange("p (h c) -> p h c", c=64),
                    bvT_sb[:].rearrange("p (h c) -> p h c", c=64))

            # ---------------- attention + proj ----------------
            # pending_proj: ah tile of the previous tt, proj emitted inside the
            # next tt's chunk loop so the PE never waits on the normalize chain.
            pending_proj = [None]

            def emit_proj(tt, ah_pair, oc):
                pr_ps = psum_s.tile([128, 512], F32, tag="ps", name="ps")
                for h in range(HEADS):
                    nc.tensor.matmul(
                        pr_ps[:],
                        projT_sb[h][:, oc * 128:(oc + 1) * 128],
                        ah_pair[h // 2][:, (h % 2) * 512:(h % 2 + 1) * 512],
                        start=(h == 0), stop=(h == HEADS - 1))
                o2 = ao.tile([128, 512], F32, tag="o2", name="o2")
                # out = (proj_psum + proj_bias) + residual, one fused DVE op
                nc.vector.scalar_tensor_tensor(
                    out=o2[:], in0=pr_ps[:], scalar=pb_sb[oc][:],
                    in1=xq_sb[oc][:, tt * 512:(tt + 1) * 512],
                    op0=mybir.AluOpType.add, op1=mybir.AluOpType.add)
                nc.sync.dma_start(
                    out_d.ap()[oc * 128:(oc + 1) * 128, tt * 512:(tt + 1) * 512],
                    o2[:])

            for tt in range(NTT):
                ah_pair = [None, None]
                for p in range(2):

                    def s_matmuls(i, p=p):
                        s_ps = psum_s.tile([128, 1024], F32, tag="ps", name="ps")
                        for u in range(2):
                            nc.tensor.matmul(
                                s_ps[:, u * 512:(u + 1) * 512],
                                k_sb[p][u * 64:(u + 1) * 64, i * 128:(i + 1) * 128],
                                q_sb[p][u * 64:(u + 1) * 64, tt * 512:(tt + 1) * 512],
                                start=True, stop=True,
                                tile_position=(u * 64, 0))
                        return s_ps

                    def emit_exp(i, s_ps):
                        """exp of one score chunk -> bf16 AP for the PV matmul."""
                        if i in ACT_CHUNKS:
                            p_t = ppool.tile([128, 1024], BF16, tag="pt", name="pt")
                            nc.scalar.activation(p_t[:], s_ps[:],
                                                 mybir.ActivationFunctionType.Exp,
                                                 scale=float(SCALE))
                            return p_t[:]
                        p16 = ppool.tile([128, 1024], I16, tag="pt", name="pt")
                        nc.vector.tensor_scalar(
                            out=p16[:], in0=s_ps[:],
                            scalar1=ALPHA, scalar2=BETA,
                            op0=mybir.AluOpType.mult, op1=mybir.AluOpType.add)
                        return p16[:].bitcast(BF16)

                    a_ps = psum_a.tile([65, 1024], F32, tag="pa", name="pa")
                    pts = {}
                    for i in range(3):
                        pts[i] = emit_exp(i, s_matmuls(i))
                    for i in range(NSC):
                        p_t = pts.pop(i)
                        for u in range(2):
                            h = 2 * p + u
                            nc.tensor.matmul(
                                a_ps[:, u * 512:(u + 1) * 512],
                                vt_sb[:, i * 260 + h * 65:i * 260 + h * 65 + 65],
                                p_t[:, u * 512:(u + 1) * 512],
                                start=(i == 0), stop=(i == NSC - 1))
                        if i + 3 < NSC:
                            pts[i + 3] = emit_exp(i + 3, s_matmuls(i + 3))
                        # stagger the previous tt's proj into the p=0 loop so
                        # its psum slots interleave with the score pipeline
                        if p == 0 and pending_proj[0] is not None and i in (2, 5):
                            emit_proj(tt - 1, pending_proj[0], 0 if i == 2 else 1)
                            if i == 5:
                                pending_proj[0] = None

                    # evict a_ps to SBUF fast (Act copy) so the single PSUM
                    # accumulator frees for the next (tt, p) iteration; the
                    # normalize chain then runs off the critical path.
                    af = afpool.tile([65, 1024], F32, tag="af", name="af")
                    nc.scalar.copy(af[:], a_ps[:])
                    d_inv = dn.tile([1, 1024], F32, tag="dinv", name="dinv")
                    nc.vector.reciprocal(d_inv[:], af[64:65, :])
                    d_bc = dn.tile([64, 1024], F32, tag="dbc", name="dbc")
                    nc.gpsimd.partition_broadcast(d_bc[:], d_inv[:])
                    ah = ahpool.tile([64, 1024], BF16, tag="ah", name="ah")
                    nc.gpsimd.tensor_mul(ah[:], af[0:64, :], d_bc[:])
                    ah_pair[p] = ah

                pending_proj[0] = ah_pair

            emit_proj(NTT - 1, pending_proj[0], 0)
            emit_proj(NTT - 1, pending_proj[0], 1)

    nc.compile()
    nc.m = get_hw_module(nc.m)
    return nc


def _host_prep(inputs):
    x = np.asarray(inputs["x"], np.float32)
    gn_w = np.asarray(inputs["gn_weight"], np.float32)
    gn_b = np.asarray(inputs["gn_bias"], np.float32)
    qkv_w = np.asarray(inputs["qkv_w"], np.float32)
    qkv_b = np.asarray(inputs["qkv_b"], np.float32)
    proj_w = np.asarray(inputs["proj_w"], np.float32)
    proj_b = np.asarray(inputs["proj_b"], np.float32)

    W_ = qkv_w * gn_w[None, :]
    b_ = qkv_w @ gn_b + qkv_b
    qkvT = np.ascontiguousarray(W_.T).astype(BF)
    # proj_w.T is [c_in(=head*d), c_out]; split head dim so each head's
    # 64 rows sit on partitions 0..63
    projT = np.ascontiguousarray(proj_w.T.reshape(HEADS, D, C)).astype(BF)

    gmat = np.zeros((128, 16), np.float32)
    gmatT = np.zeros((16, 128), np.float32)
    for ch in range(128):
        gmat[ch, ch // GS] = 1.0 / GS
        gmatT[ch // GS, ch] = 1.0
    shared = {
        "qkvT": qkvT,
        "qkvb": b_.reshape(3 * C, 1).astype(np.float32),
        "projT": projT,
        "projb": proj_b.reshape(C, 1).astype(np.float32),
        "gmat": gmat,
        "gmatT": gmatT,
    }
    x3 = x.reshape(B, C, T)
    in_maps = []
    for j in range(8):
        b, hf = j // 2, j % 2
        m = dict(shared)
        m["xb"] = np.ascontiguousarray(x3[b])
        m["xq"] = np.ascontiguousarray(x3[b][:, hf * TQ:(hf + 1) * TQ])
        in_maps.append(m)
    return x, in_maps


def kernel(**inputs) -> np.ndarray:
    if "nc" not in _CACHED:
        _CACHED["nc"] = _build()
    nc = _CACHED["nc"]
    x, in_maps = _host_prep(inputs)
    res = bass_utils.run_bass_kernel_spmd(nc, in_maps, core_ids=list(range(8)))
    out = np.zeros((B, C, T), np.float32)
    for j in range(8):
        b, hf = j // 2, j % 2
        out[b][:, hf * TQ:(hf + 1) * TQ] = np.asarray(
            res.results[j]["out"], np.float32)
    return out.reshape(B, C, Himg, Wimg)


# revision 8
# speedup vs baseline: 1.1802x; 1.1407x over previous
"""AttentionBlock (GroupNorm -> qkv conv1x1 -> 4-head attention -> proj -> residual)
as a distributed Bass/Tile kernel on 8 TRN2 NeuronCores.

Sharding: core j handles batch b = j//2 and query-half h = j%2 (2048 of the 4096
spatial positions). K/V are computed full-length per core (cheap), so the proj
output slices are disjoint across cores and no collectives are needed.

Engine split: exp of the attention scores is the hard bottleneck (only the Act
engine has exp, 128 lanes @ 1.2 GHz). So ~2/3 of the score chunks use a
Schraudolph-style exp approximation on the Vector engine (one tensor_scalar
producing int16 bits that reinterpret as bf16), the rest use exact exp on Act.
Denominator broadcast + normalize run on GPSIMD; GN apply runs on Act.

Self-contained: hardcodes all shapes; host side only reshapes/shards inputs,
transposes/folds weights, and reassembles the 8 output slices.
"""
import numpy as np
import ml_dtypes

import concourse.bass as bass
import concourse.bacc as bacc
import concourse.tile as tile
from concourse import mybir
from concourse import bass_utils
from concourse.bass_interp import get_hw_module

F32 = mybir.dt.float32
BF16 = mybir.dt.bfloat16
I16 = mybir.dt.int16
BF = ml_dtypes.bfloat16

B, C, Himg, Wimg = 4, 256, 64, 64
T = Himg * Wimg            # 4096 tokens
HEADS, D = 4, 64           # 4 heads x 64 dims
GROUPS, GS = 32, 8         # groupnorm: 32 groups of 8 channels
EPS = 1e-5
TQ = T // 2                # queries per core (2048)
NTT = TQ // 512            # query tiles of 512
NSC = T // 128             # key chunks of 128
SCALE = 1.0 / np.sqrt(D)

# Schraudolph exp-approx constants (bf16 bit trick):
#   bits16 = round(ALPHA * s + BETA);  bits16 viewed as bf16 ~ exp(SCALE * s)
# ALPHA is folded into the K weights host-side, so the kernel's score psum
# already holds ALPHA * s and the DVE exp is a single scalar-add.
# BETA tuned so the approximation is unbiased (E[approx/exp] = 1), which makes
# mixing approx (DVE) and exact (Act) chunks inside one softmax safe.
ALPHA = float(128.0 * np.log2(np.e) * SCALE)
BETA = float(127.0 * 128.0 - 7.5)
# exact-exp scale for the Act engine given the pre-scaled scores
ACT_SCALE = float(np.log(2.0) / 128.0)
# chunks of each (tt, p) iteration computed on the DVE (approx); the rest use
# exact exp on Act. Act is the faster engine per element, so it gets more.
DVE_CHUNKS = frozenset(i for i in range(NSC) if i % 5 in (1, 3))

_CACHED = {}


def _build():
    nc = bacc.Bacc("TRN2", target_bir_lowering=False, debug=False,
                   enable_asserts=False, num_devices=8)

    xb_d = nc.dram_tensor("xb", [C, T], F32, kind="ExternalInput")
    xq_d = nc.dram_tensor("xq", [C, TQ], F32, kind="ExternalInput")
    qkvT_d = nc.dram_tensor("qkvT", [C, 3 * C], BF16, kind="ExternalInput")
    qkvb_d = nc.dram_tensor("qkvb", [3 * C, 1], F32, kind="ExternalInput")
    projT_d = nc.dram_tensor("projT", [HEADS, D, C], BF16, kind="ExternalInput")
    projb_d = nc.dram_tensor("projb", [C, 1], F32, kind="ExternalInput")
    gmat_d = nc.dram_tensor("gmat", [128, 16], F32, kind="ExternalInput")
    gmatT_d = nc.dram_tensor("gmatT", [16, 128], F32, kind="ExternalInput")
    out_d = nc.dram_tensor("out", [C, TQ], F32, kind="ExternalOutput")

    with tile.TileContext(nc) as tc:
        with (
            tc.tile_pool(name="consts", bufs=1) as consts,
            tc.tile_pool(name="data", bufs=1) as data,
            tc.tile_pool(name="gn", bufs=1) as gn,
            tc.tile_pool(name="pt", bufs=4) as ppool,
            tc.tile_pool(name="dn", bufs=2) as dn,
            tc.tile_pool(name="af", bufs=2) as afpool,
            tc.tile_pool(name="ao", bufs=4) as ao,
            tc.tile_pool(name="ah", bufs=4) as ahpool,
            tc.tile_pool(name="ps", bufs=3, space="PSUM") as psum_s,
            tc.tile_pool(name="pa", bufs=1, space="PSUM") as psum_a,
        ):
            # ---------------- constant / weight loads ----------------
            qkvT_sb, qb_sb, kb_sb, pb_sb = [], [], [], []
            for ct in range(2):
                w = consts.tile([128, 3 * C], BF16, tag=f"qkvT{ct}", name=f"qkvT{ct}")
                nc.sync.dma_start(w[:], qkvT_d.ap()[ct * 128:(ct + 1) * 128, :])
                qkvT_sb.append(w)
                qb = consts.tile([128, 1], F32, tag=f"qb{ct}", name=f"qb{ct}")
                nc.sync.dma_start(qb[:], qkvb_d.ap()[ct * 128:(ct + 1) * 128, :])
                qb_sb.append(qb)
                kb = consts.tile([128, 1], F32, tag=f"kb{ct}", name=f"kb{ct}")
                nc.sync.dma_start(kb[:], qkvb_d.ap()[C + ct * 128:C + (ct + 1) * 128, :])
                kb_sb.append(kb)
                pb = consts.tile([128, 1], F32, tag=f"pb{ct}", name=f"pb{ct}")
                nc.sync.dma_start(pb[:], projb_d.ap()[ct * 128:(ct + 1) * 128, :])
                pb_sb.append(pb)
            projT_sb = []
            for h in range(HEADS):
                pw = consts.tile([D, C], BF16, tag=f"projT{h}", name=f"projT{h}")
                nc.sync.dma_start(pw[:], projT_d.ap()[h])
                projT_sb.append(pw)
            gmat_sb = consts.tile([128, 16], F32, tag="gmat", name="gmat")
            nc.sync.dma_start(gmat_sb[:], gmat_d.ap()[:])
            gmatT_sb = consts.tile([16, 128], F32, tag="gmatT", name="gmatT")
            nc.sync.dma_start(gmatT_sb[:], gmatT_d.ap()[:])
            # v-bias broadcast along partitions: [128, 256] from qkvb[512:768]
            bvT_sb = consts.tile([128, C], F32, tag="bvT", name="bvT")
            bvT_src = bass.AP(tensor=qkvb_d, offset=2 * C, ap=[[0, 128], [1, C]])
            nc.sync.dma_start(bvT_sb[:], bvT_src)
            eps_t = gn.tile([16, 1], F32, tag="eps", name="eps")
            nc.vector.memset(eps_t[:], EPS)

            # ---------------- x loads (column-split so stats start early) ----
            xb_sb, xq_sb = [], []
            for ct in range(2):
                xt = data.tile([128, T], F32, tag=f"xb{ct}", name=f"xb{ct}")
                for t8 in range(8):
                    nc.sync.dma_start(
                        xt[:, t8 * 512:(t8 + 1) * 512],
                        xb_d.ap()[ct * 128:(ct + 1) * 128, t8 * 512:(t8 + 1) * 512])
                xb_sb.append(xt)
                xqt = data.tile([128, TQ], F32, tag=f"xq{ct}", name=f"xq{ct}")
                for t4 in range(4):
                    nc.sync.dma_start(
                        xqt[:, t4 * 512:(t4 + 1) * 512],
                        xq_d.ap()[ct * 128:(ct + 1) * 128, t4 * 512:(t4 + 1) * 512])
                xq_sb.append(xqt)

            # ---------------- GroupNorm statistics ----------------
            # per-channel mean/var via bn_stats, then 8-channel group
            # aggregation via tiny PE matmuls with the group matrices.
            stats2 = []
            for ct in range(2):
                st = gn.tile([128, 8, 6], F32, tag=f"st{ct}", name=f"st{ct}")
                for i in range(8):
                    nc.vector.bn_stats(st[:, i, :], xb_sb[ct][:, i * 512:(i + 1) * 512])
                mv = gn.tile([128, 2], F32, tag=f"mv{ct}", name=f"mv{ct}")
                nc.vector.bn_aggr(mv[:], st[:])
                s2 = gn.tile([128, 2], F32, tag=f"s2{ct}", name=f"s2{ct}")
                nc.vector.tensor_copy(s2[:, 0:1], mv[:, 0:1])
                m2 = gn.tile([128, 1], F32, tag=f"m2{ct}", name=f"m2{ct}")
                nc.vector.tensor_mul(m2[:], mv[:, 0:1], mv[:, 0:1])
                nc.vector.tensor_add(s2[:, 1:2], m2[:], mv[:, 1:2])
                stats2.append(s2)

            # group (mean, E[x^2]) per channel tile -> [16, 2] each
            gs_ps, bc_sb = [], []
            vg = gn.tile([16, 2], F32, tag="vg", name="vg")
            for ct in range(2):
                g1 = psum_s.tile([16, 2], F32, tag="ps", name="ps")
                nc.tensor.matmul(g1[:], gmat_sb[:], stats2[ct][:],
                                 start=True, stop=True)
                gsb = gn.tile([16, 2], F32, tag=f"gsb{ct}", name=f"gsb{ct}")
                nc.vector.tensor_copy(gsb[:], g1[:])
                gs_ps.append(gsb)
                m2g = gn.tile([16, 1], F32, tag=f"m2g{ct}", name=f"m2g{ct}")
                nc.vector.tensor_mul(m2g[:], gsb[:, 0:1], gsb[:, 0:1])
                nc.vector.tensor_sub(vg[:, ct:ct + 1], gsb[:, 1:2], m2g[:])
            sg = gn.tile([16, 2], F32, tag="sg", name="sg")
            nc.scalar.activation(sg[:], vg[:], mybir.ActivationFunctionType.Sqrt,
                                 bias=eps_t[:])
            rg = gn.tile([16, 2], F32, tag="rg", name="rg")
            nc.vector.reciprocal(rg[:], sg[:])
            negmr_sb = []
            for ct in range(2):
                bcv = gn.tile([16, 2], F32, tag=f"bcv{ct}", name=f"bcv{ct}")
                nc.vector.tensor_copy(bcv[:, 0:1], gs_ps[ct][:, 0:1])
                nc.vector.tensor_copy(bcv[:, 1:2], rg[:, ct:ct + 1])
                b1 = psum_s.tile([128, 2], F32, tag="ps", name="ps")
                nc.tensor.matmul(b1[:], gmatT_sb[:], bcv[:],
                                 start=True, stop=True)
                bsb = gn.tile([128, 2], F32, tag=f"bc{ct}", name=f"bc{ct}")
                nc.vector.tensor_copy(bsb[:], b1[:])
                bc_sb.append(bsb)
                # -mean * rstd, for the fused (x*r + b) GN apply on Act
                nmr = gn.tile([128, 1], F32, tag=f"nmr{ct}", name=f"nmr{ct}")
                nc.vector.scalar_tensor_tensor(
                    out=nmr[:], in0=bsb[:, 0:1], scalar=-1.0, in1=bsb[:, 1:2],
                    op0=mybir.AluOpType.mult, op1=mybir.AluOpType.mult)
                negmr_sb.append(nmr)

            # ---------------- apply GN -> xn (bf16), on GPSIMD --------------
            # (keeps Act + DVE free for the exp storm; gpsimd reads SBUF only)
            xn_sb, xnq_sb = [], []
            for ct in range(2):
                xn = data.tile([128, T], BF16, tag=f"xn{ct}", name=f"xn{ct}")
                for t8 in range(4):
                    nc.gpsimd.tensor_scalar(
                        out=xn[:, t8 * 1024:(t8 + 1) * 1024],
                        in0=xb_sb[ct][:, t8 * 1024:(t8 + 1) * 1024],
                        scalar1=bc_sb[ct][:, 1:2], scalar2=negmr_sb[ct][:],
                        op0=mybir.AluOpType.mult, op1=mybir.AluOpType.add)
                xn_sb.append(xn)
                xnq = data.tile([128, TQ], BF16, tag=f"xnq{ct}", name=f"xnq{ct}")
                for t4 in range(2):
                    nc.gpsimd.tensor_scalar(
                        out=xnq[:, t4 * 1024:(t4 + 1) * 1024],
                        in0=xq_sb[ct][:, t4 * 1024:(t4 + 1) * 1024],
                        scalar1=bc_sb[ct][:, 1:2], scalar2=negmr_sb[ct][:],
                        op0=mybir.AluOpType.mult, op1=mybir.AluOpType.add)
                xnq_sb.append(xnq)

            # ---------------- K (full length), Q (this half) ----------------
            k_sb = [data.tile([128, T], BF16, tag=f"k{p}", name=f"k{p}")
                    for p in range(2)]
            for p in range(2):
                for t8 in range(8):
                    kv_ps = psum_s.tile([128, 512], F32, tag="ps", name="ps")
                    for ct in range(2):
                        nc.tensor.matmul(
                            kv_ps[:],
                            qkvT_sb[ct][:, C + p * 128:C + (p + 1) * 128],
                            xn_sb[ct][:, t8 * 512:(t8 + 1) * 512],
                            start=(ct == 0), stop=(ct == 1))
                    nc.vector.tensor_scalar_add(
                        out=k_sb[p][:, t8 * 512:(t8 + 1) * 512],
                        in0=kv_ps[:], scalar1=kb_sb[p][:])

            q_sb = [data.tile([128, TQ], BF16, tag=f"q{p}", name=f"q{p}")
                    for p in range(2)]
            for p in range(2):
                for t4 in range(NTT):
                    q_ps = psum_s.tile([128, 512], F32, tag="ps", name="ps")
                    for ct in range(2):
                        nc.tensor.matmul(
                            q_ps[:],
                            qkvT_sb[ct][:, p * 128:(p + 1) * 128],
                            xnq_sb[ct][:, t4 * 512:(t4 + 1) * 512],
                            start=(ct == 0), stop=(ct == 1))
                    nc.vector.tensor_scalar_add(
                        out=q_sb[p][:, t4 * 512:(t4 + 1) * 512],
                        in0=q_ps[:], scalar1=qb_sb[p][:])

            # ---------------- vT: [s, head*65] with ones column per head ------
            # vt[:, i*260 + h*65 + j] = V[h*64+j, i*128:...]^T ; col h*65+64 == 1
            vt_sb = data.tile([128, NSC * 260], BF16, tag="vt", name="vt")
            ones_cols = vt_sb[:].rearrange("p (s h c) -> p s h c", s=NSC, c=65)
            nc.vector.memset(ones_cols[:, :, :, 64:65], 1.0)
            for i in range(NSC):
                vt_ps = psum_s.tile([128, C], F32, tag="ps", name="ps")
                for ct in range(2):
                    nc.tensor.matmul(
                        vt_ps[:],
                        xn_sb[ct][:, i * 128:(i + 1) * 128],
                        qkvT_sb[ct][:, 2 * C:3 * C],
                        start=(ct == 0), stop=(ct == 1))
                dst = vt_sb[:, i * 260:(i + 1) * 260].rearrange(
                    "p (h c) -> p h c", c=65)[:, :, 0:64]
                nc.vector.tensor_add(
                    dst,
                    vt_ps[:].rearrange("p (h c) -> p h c", c=64),
                    bvT_sb[:].rearrange("p (h c) -> p h c", c=64))

            # ---------------- attention + proj ----------------
            # pending_proj: ah tile of the previous tt, proj emitted inside the
            # next tt's chunk loop so the PE never waits on the normalize chain.
            pending_proj = [None]

            def emit_proj(tt, ah_pair, oc):
                pr_ps = psum_s.tile([128, 512], F32, tag="ps", name="ps")
                for h in range(HEADS):
                    nc.tensor.matmul(
                        pr_ps[:],
                        projT_sb[h][:, oc * 128:(oc + 1) * 128],
                        ah_pair[h // 2][:, (h % 2) * 512:(h % 2 + 1) * 512],
                        start=(h == 0), stop=(h == HEADS - 1))
                o2 = ao.tile([128, 512], F32, tag="o2", name="o2")
                # out = (proj_psum + proj_bias) + residual, one fused DVE op
                nc.vector.scalar_tensor_tensor(
                    out=o2[:], in0=pr_ps[:], scalar=pb_sb[oc][:],
                    in1=xq_sb[oc][:, tt * 512:(tt + 1) * 512],
                    op0=mybir.AluOpType.add, op1=mybir.AluOpType.add)
                nc.sync.dma_start(
                    out_d.ap()[oc * 128:(oc + 1) * 128, tt * 512:(tt + 1) * 512],
                    o2[:])

            for tt in range(NTT):
                ah_pair = [None, None]
                for p in range(2):

                    def s_matmuls(i, p=p):
                        s_ps = psum_s.tile([128, 1024], F32, tag="ps", name="ps")
                        for u in range(2):
                            nc.tensor.matmul(
                                s_ps[:, u * 512:(u + 1) * 512],
                                k_sb[p][u * 64:(u + 1) * 64, i * 128:(i + 1) * 128],
                                q_sb[p][u * 64:(u + 1) * 64, tt * 512:(tt + 1) * 512],
                                start=True, stop=True,
                                tile_position=(u * 64, 0))
                        return s_ps

                    def emit_exp(i, s_ps):
                        """exp of one score chunk -> bf16 AP for the PV matmul."""
                        if i not in DVE_CHUNKS:
                            p_t = ppool.tile([128, 1024], BF16, tag="pt", name="pt")
                            nc.scalar.activation(p_t[:], s_ps[:],
                                                 mybir.ActivationFunctionType.Exp,
                                                 scale=ACT_SCALE)
                            return p_t[:]
                        p16 = ppool.tile([128, 1024], I16, tag="pt", name="pt")
                        nc.vector.tensor_scalar_add(
                            out=p16[:], in0=s_ps[:], scalar1=BETA)
                        return p16[:].bitcast(BF16)

                    a_ps = psum_a.tile([65, 1024], F32, tag="pa", name="pa")
                    pts = {}
                    for i in range(3):
                        pts[i] = emit_exp(i, s_matmuls(i))
                    for i in range(NSC):
                        p_t = pts.pop(i)
                        for u in range(2):
                            h = 2 * p + u
                            nc.tensor.matmul(
                                a_ps[:, u * 512:(u + 1) * 512],
                                vt_sb[:, i * 260 + h * 65:i * 260 + h * 65 + 65],
                                p_t[:, u * 512:(u + 1) * 512],
                                start=(i == 0), stop=(i == NSC - 1))
                        if i + 3 < NSC:
                            pts[i + 3] = emit_exp(i + 3, s_matmuls(i + 3))
                        # stagger the previous tt's proj into the p=0 loop so
                        # its psum slots interleave with the score pipeline
                        if p == 0 and pending_proj[0] is not None and i in (2, 5):
                            emit_proj(tt - 1, pending_proj[0], 0 if i == 2 else 1)
                            if i == 5:
                                pending_proj[0] = None

                    # evict a_ps to SBUF fast (Act copy) so the single PSUM
                    # accumulator frees for the next (tt, p) iteration; the
                    # normalize chain then runs off the critical path.
                    af = afpool.tile([65, 1024], F32, tag="af", name="af")
                    nc.scalar.copy(af[:], a_ps[:])
                    # custom ops (recip_approx, partition_broadcast) ignore the
                    # AP base partition, so first move the den row to a
                    # partition-0 tile with an ordinary copy.
                    den0 = dn.tile([1, 1024], F32, tag="den0", name="den0")
                    nc.vector.tensor_copy(den0[:], af[64:65, :])
                    d_inv = dn.tile([1, 1024], F32, tag="dinv", name="dinv")
                    nc.vector.reciprocal_approx_fast(d_inv[:], den0[:])
                    d_bc = dn.tile([64, 1024], F32, tag="dbc", name="dbc")
                    nc.gpsimd.partition_broadcast(d_bc[:], d_inv[:])
                    ah = ahpool.tile([64, 1024], BF16, tag="ah", name="ah")
                    nc.gpsimd.tensor_mul(ah[:], af[0:64, :], d_bc[:])
                    ah_pair[p] = ah

                pending_proj[0] = ah_pair

            emit_proj(NTT - 1, pending_proj[0], 0)
            emit_proj(NTT - 1, pending_proj[0], 1)

    nc.compile()
    nc.m = get_hw_module(nc.m)
    return nc


def _host_prep(inputs):
    x = np.asarray(inputs["x"], np.float32)
    gn_w = np.asarray(inputs["gn_weight"], np.float32)
    gn_b = np.asarray(inputs["gn_bias"], np.float32)
    qkv_w = np.asarray(inputs["qkv_w"], np.float32)
    qkv_b = np.asarray(inputs["qkv_b"], np.float32)
    proj_w = np.asarray(inputs["proj_w"], np.float32)
    proj_b = np.asarray(inputs["proj_b"], np.float32)

    W_ = qkv_w * gn_w[None, :]
    b_ = qkv_w @ gn_b + qkv_b
    # fold the Schraudolph ALPHA into the K projection so the score psum
    # arrives pre-scaled (ALPHA * q.k) and the DVE exp is a single add
    W_[C:2 * C, :] *= ALPHA
    b_[C:2 * C] *= ALPHA
    qkvT = np.ascontiguousarray(W_.T).astype(BF)
    # proj_w.T is [c_in(=head*d), c_out]; split head dim so each head's
    # 64 rows sit on partitions 0..63
    projT = np.ascontiguousarray(proj_w.T.reshape(HEADS, D, C)).astype(BF)

    gmat = np.zeros((128, 16), np.float32)
    gmatT = np.zeros((16, 128), np.float32)
    for ch in range(128):
        gmat[ch, ch // GS] = 1.0 / GS
        gmatT[ch // GS, ch] = 1.0
    shared = {
        "qkvT": qkvT,
        "qkvb": b_.reshape(3 * C, 1).astype(np.float32),
        "projT": projT,
        "projb": proj_b.reshape(C, 1).astype(np.float32),
        "gmat": gmat,
        "gmatT": gmatT,
    }
    x3 = x.reshape(B, C, T)
    in_maps = []
    for j in range(8):
        b, hf = j // 2, j % 2
        m = dict(shared)
        m["xb"] = np.ascontiguousarray(x3[b])
        m["xq"] = np.ascontiguousarray(x3[b][:, hf * TQ:(hf + 1) * TQ])
        in_maps.append(m)
    return x, in_maps


def kernel(**inputs) -> np.ndarray:
    if "nc" not in _CACHED:
        _CACHED["nc"] = _build()
    nc = _CACHED["nc"]
    x, in_maps = _host_prep(inputs)
    res = bass_utils.run_bass_kernel_spmd(nc, in_maps, core_ids=list(range(8)))
    out = np.zeros((B, C, T), np.float32)
    for j in range(8):
        b, hf = j // 2, j % 2
        out[b][:, hf * TQ:(hf + 1) * TQ] = np.asarray(
            res.results[j]["out"], np.float32)
    return out.reshape(B, C, Himg, Wimg)


# revision 11
# speedup vs baseline: 1.2482x; 1.0576x over previous
"""AttentionBlock (GroupNorm -> qkv conv1x1 -> 4-head attention -> proj -> residual)
as a distributed Bass/Tile kernel on 8 TRN2 NeuronCores.

Sharding: core j handles batch b = j//2 and query-half h = j%2 (2048 of the 4096
spatial positions). K/V are computed full-length per core (cheap), so the proj
output slices are disjoint across cores and no collectives are needed.

Engine split: exp of the attention scores is the hard bottleneck (only the Act
engine has exp, 128 lanes @ 1.2 GHz). So ~2/3 of the score chunks use a
Schraudolph-style exp approximation on the Vector engine (one tensor_scalar
producing int16 bits that reinterpret as bf16), the rest use exact exp on Act.
Denominator broadcast + normalize run on GPSIMD; GN apply runs on Act.

Self-contained: hardcodes all shapes; host side only reshapes/shards inputs,
transposes/folds weights, and reassembles the 8 output slices.
"""
import numpy as np
import ml_dtypes

import concourse.bass as bass
import concourse.bacc as bacc
import concourse.tile as tile
from concourse import mybir
from concourse import bass_utils
from concourse.bass_interp import get_hw_module

F32 = mybir.dt.float32
BF16 = mybir.dt.bfloat16
I16 = mybir.dt.int16
BF = ml_dtypes.bfloat16

B, C, Himg, Wimg = 4, 256, 64, 64
T = Himg * Wimg            # 4096 tokens
HEADS, D = 4, 64           # 4 heads x 64 dims
GROUPS, GS = 32, 8         # groupnorm: 32 groups of 8 channels
EPS = 1e-5
TQ = T // 2                # queries per core (2048)
NTT = TQ // 512            # query tiles of 512
NSC = T // 128             # key chunks of 128
SCALE = 1.0 / np.sqrt(D)

# Schraudolph exp-approx constants (bf16 bit trick):
#   bits16 = round(ALPHA * s + BETA);  bits16 viewed as bf16 ~ exp(SCALE * s)
# ALPHA is folded into the K weights host-side, so the kernel's score psum
# already holds ALPHA * s and the DVE exp is a single scalar-add.
# BETA tuned so the approximation is unbiased (E[approx/exp] = 1), which makes
# mixing approx (DVE) and exact (Act) chunks inside one softmax safe.
ALPHA = float(128.0 * np.log2(np.e) * SCALE)
BETA = float(127.0 * 128.0 - 7.5)
# exact-exp scale for the Act engine given the pre-scaled scores
ACT_SCALE = float(np.log(2.0) / 128.0)
# chunks of each (tt, p) iteration computed on the DVE (approx); the rest use
# exact exp on Act. Perfect alternation keeps both engine queues shallow, and
# ending on a DVE chunk leaves Act free for the a_ps eviction copy.
DVE_CHUNKS = frozenset(i for i in range(NSC) if i % 2 == 1)

_CACHED = {}


def _build():
    nc = bacc.Bacc("TRN2", target_bir_lowering=False, debug=False,
                   enable_asserts=False, num_devices=8)

    xb_d = nc.dram_tensor("xb", [C, T], F32, kind="ExternalInput")
    xq_d = nc.dram_tensor("xq", [C, TQ], F32, kind="ExternalInput")
    qkvT_d = nc.dram_tensor("qkvT", [C, 3 * C], BF16, kind="ExternalInput")
    qkvb_d = nc.dram_tensor("qkvb", [3 * C, 1], F32, kind="ExternalInput")
    projT_d = nc.dram_tensor("projT", [HEADS, D, C], BF16, kind="ExternalInput")
    projb_d = nc.dram_tensor("projb", [C, 1], F32, kind="ExternalInput")
    gmat_d = nc.dram_tensor("gmat", [128, 16], F32, kind="ExternalInput")
    gmatT_d = nc.dram_tensor("gmatT", [16, 128], F32, kind="ExternalInput")
    out_d = nc.dram_tensor("out", [C, TQ], F32, kind="ExternalOutput")

    with tile.TileContext(nc) as tc:
        with (
            tc.tile_pool(name="consts", bufs=1) as consts,
            tc.tile_pool(name="data", bufs=1) as data,
            tc.tile_pool(name="gn", bufs=1) as gn,
            tc.tile_pool(name="pt", bufs=4) as ppool,
            tc.tile_pool(name="dn", bufs=2) as dn,
            tc.tile_pool(name="af", bufs=2) as afpool,
            tc.tile_pool(name="ao", bufs=4) as ao,
            tc.tile_pool(name="ah", bufs=4) as ahpool,
            tc.tile_pool(name="ps", bufs=3, space="PSUM") as psum_s,
            tc.tile_pool(name="pa", bufs=1, space="PSUM") as psum_a,
        ):
            # ---------------- constant / weight loads ----------------
            qkvT_sb, qb_sb, kb_sb, pb_sb = [], [], [], []
            for ct in range(2):
                w = consts.tile([128, 3 * C], BF16, tag=f"qkvT{ct}", name=f"qkvT{ct}")
                nc.sync.dma_start(w[:], qkvT_d.ap()[ct * 128:(ct + 1) * 128, :])
                qkvT_sb.append(w)
                qb = consts.tile([128, 1], F32, tag=f"qb{ct}", name=f"qb{ct}")
                nc.sync.dma_start(qb[:], qkvb_d.ap()[ct * 128:(ct + 1) * 128, :])
                qb_sb.append(qb)
                kb = consts.tile([128, 1], F32, tag=f"kb{ct}", name=f"kb{ct}")
                nc.sync.dma_start(kb[:], qkvb_d.ap()[C + ct * 128:C + (ct + 1) * 128, :])
                kb_sb.append(kb)
                pb = consts.tile([128, 1], F32, tag=f"pb{ct}", name=f"pb{ct}")
                nc.sync.dma_start(pb[:], projb_d.ap()[ct * 128:(ct + 1) * 128, :])
                pb_sb.append(pb)
            projT_sb = []
            for h in range(HEADS):
                pw = consts.tile([D, C], BF16, tag=f"projT{h}", name=f"projT{h}")
                nc.sync.dma_start(pw[:], projT_d.ap()[h])
                projT_sb.append(pw)
            gmat_sb = consts.tile([128, 16], F32, tag="gmat", name="gmat")
            nc.sync.dma_start(gmat_sb[:], gmat_d.ap()[:])
            gmatT_sb = consts.tile([16, 128], F32, tag="gmatT", name="gmatT")
            nc.sync.dma_start(gmatT_sb[:], gmatT_d.ap()[:])
            # v-bias broadcast along partitions: [128, 256] from qkvb[512:768]
            bvT_sb = consts.tile([128, C], F32, tag="bvT", name="bvT")
            bvT_src = bass.AP(tensor=qkvb_d, offset=2 * C, ap=[[0, 128], [1, C]])
            nc.sync.dma_start(bvT_sb[:], bvT_src)
            eps_t = gn.tile([16, 1], F32, tag="eps", name="eps")
            nc.vector.memset(eps_t[:], EPS)

            # ---------------- x loads (column-split so stats start early) ----
            xb_sb, xq_sb = [], []
            for ct in range(2):
                xt = data.tile([128, T], F32, tag=f"xb{ct}", name=f"xb{ct}")
                for t8 in range(8):
                    nc.sync.dma_start(
                        xt[:, t8 * 512:(t8 + 1) * 512],
                        xb_d.ap()[ct * 128:(ct + 1) * 128, t8 * 512:(t8 + 1) * 512])
                xb_sb.append(xt)
                xqt = data.tile([128, TQ], F32, tag=f"xq{ct}", name=f"xq{ct}")
                for t4 in range(4):
                    nc.sync.dma_start(
                        xqt[:, t4 * 512:(t4 + 1) * 512],
                        xq_d.ap()[ct * 128:(ct + 1) * 128, t4 * 512:(t4 + 1) * 512])
                xq_sb.append(xqt)

            # ---------------- GroupNorm statistics ----------------
            # per-channel mean/var via bn_stats, then 8-channel group
            # aggregation via tiny PE matmuls with the group matrices.
            stats2 = []
            for ct in range(2):
                st = gn.tile([128, 8, 6], F32, tag=f"st{ct}", name=f"st{ct}")
                for i in range(8):
                    nc.vector.bn_stats(st[:, i, :], xb_sb[ct][:, i * 512:(i + 1) * 512])
                mv = gn.tile([128, 2], F32, tag=f"mv{ct}", name=f"mv{ct}")
                nc.vector.bn_aggr(mv[:], st[:])
                s2 = gn.tile([128, 2], F32, tag=f"s2{ct}", name=f"s2{ct}")
                nc.vector.tensor_copy(s2[:, 0:1], mv[:, 0:1])
                m2 = gn.tile([128, 1], F32, tag=f"m2{ct}", name=f"m2{ct}")
                nc.vector.tensor_mul(m2[:], mv[:, 0:1], mv[:, 0:1])
                nc.vector.tensor_add(s2[:, 1:2], m2[:], mv[:, 1:2])
                stats2.append(s2)

            # group (mean, E[x^2]) per channel tile -> [16, 2] each
            gs_ps, bc_sb = [], []
            vg = gn.tile([16, 2], F32, tag="vg", name="vg")
            for ct in range(2):
                g1 = psum_s.tile([16, 2], F32, tag="ps", name="ps")
                nc.tensor.matmul(g1[:], gmat_sb[:], stats2[ct][:],
                                 start=True, stop=True)
                gsb = gn.tile([16, 2], F32, tag=f"gsb{ct}", name=f"gsb{ct}")
                nc.vector.tensor_copy(gsb[:], g1[:])
                gs_ps.append(gsb)
                m2g = gn.tile([16, 1], F32, tag=f"m2g{ct}", name=f"m2g{ct}")
                nc.vector.tensor_mul(m2g[:], gsb[:, 0:1], gsb[:, 0:1])
                nc.vector.tensor_sub(vg[:, ct:ct + 1], gsb[:, 1:2], m2g[:])
            sg = gn.tile([16, 2], F32, tag="sg", name="sg")
            nc.scalar.activation(sg[:], vg[:], mybir.ActivationFunctionType.Sqrt,
                                 bias=eps_t[:])
            rg = gn.tile([16, 2], F32, tag="rg", name="rg")
            nc.vector.reciprocal(rg[:], sg[:])
            negmr_sb = []
            for ct in range(2):
                bcv = gn.tile([16, 2], F32, tag=f"bcv{ct}", name=f"bcv{ct}")
                nc.vector.tensor_copy(bcv[:, 0:1], gs_ps[ct][:, 0:1])
                nc.vector.tensor_copy(bcv[:, 1:2], rg[:, ct:ct + 1])
                b1 = psum_s.tile([128, 2], F32, tag="ps", name="ps")
                nc.tensor.matmul(b1[:], gmatT_sb[:], bcv[:],
                                 start=True, stop=True)
                bsb = gn.tile([128, 2], F32, tag=f"bc{ct}", name=f"bc{ct}")
                nc.vector.tensor_copy(bsb[:], b1[:])
                bc_sb.append(bsb)
                # -mean * rstd, for the fused (x*r + b) GN apply on Act
                nmr = gn.tile([128, 1], F32, tag=f"nmr{ct}", name=f"nmr{ct}")
                nc.vector.scalar_tensor_tensor(
                    out=nmr[:], in0=bsb[:, 0:1], scalar=-1.0, in1=bsb[:, 1:2],
                    op0=mybir.AluOpType.mult, op1=mybir.AluOpType.mult)
                negmr_sb.append(nmr)

            # ---------------- apply GN -> xn (bf16), on GPSIMD --------------
            # (keeps Act + DVE free for the exp storm; gpsimd reads SBUF only)
            xn_sb, xnq_sb = [], []
            for ct in range(2):
                xn = data.tile([128, T], BF16, tag=f"xn{ct}", name=f"xn{ct}")
                for t8 in range(4):
                    nc.gpsimd.tensor_scalar(
                        out=xn[:, t8 * 1024:(t8 + 1) * 1024],
                        in0=xb_sb[ct][:, t8 * 1024:(t8 + 1) * 1024],
                        scalar1=bc_sb[ct][:, 1:2], scalar2=negmr_sb[ct][:],
                        op0=mybir.AluOpType.mult, op1=mybir.AluOpType.add)
                xn_sb.append(xn)
                xnq = data.tile([128, TQ], BF16, tag=f"xnq{ct}", name=f"xnq{ct}")
                for t4 in range(2):
                    nc.gpsimd.tensor_scalar(
                        out=xnq[:, t4 * 1024:(t4 + 1) * 1024],
                        in0=xq_sb[ct][:, t4 * 1024:(t4 + 1) * 1024],
                        scalar1=bc_sb[ct][:, 1:2], scalar2=negmr_sb[ct][:],
                        op0=mybir.AluOpType.mult, op1=mybir.AluOpType.add)
                xnq_sb.append(xnq)

            # ---------------- PE warm-up burst ----------------
            # ~4us of throwaway matmuls gated on the first xn chunk, so the
            # HAM clock gate reaches K=8/8 right as the real qkv matmuls
            # start (otherwise the whole qkv phase runs at 1.2 GHz).
            for w in range(18):
                junk_ps = psum_s.tile([128, 512], F32, tag="ps", name="ps")
                nc.tensor.matmul(
                    junk_ps[:], qkvT_sb[0][:, 0:128],
                    xn_sb[0][:, 0:512], start=True, stop=True)

            # ---------------- K (full length), Q (this half) ----------------
            # psum->sbuf copies (+ per-partition bias) run on the Act engine,
            # which is otherwise idle here; DVE keeps the vt adds below.
            k_sb = [data.tile([128, T], BF16, tag=f"k{p}", name=f"k{p}")
                    for p in range(2)]
            for p in range(2):
                for t8 in range(8):
                    kv_ps = psum_s.tile([128, 512], F32, tag="ps", name="ps")
                    for ct in range(2):
                        nc.tensor.matmul(
                            kv_ps[:],
                            qkvT_sb[ct][:, C + p * 128:C + (p + 1) * 128],
                            xn_sb[ct][:, t8 * 512:(t8 + 1) * 512],
                            start=(ct == 0), stop=(ct == 1))
                    nc.scalar.activation(
                        k_sb[p][:, t8 * 512:(t8 + 1) * 512], kv_ps[:],
                        mybir.ActivationFunctionType.Identity,
                        bias=kb_sb[p][:])

            q_sb = [data.tile([128, TQ], BF16, tag=f"q{p}", name=f"q{p}")
                    for p in range(2)]
            for p in range(2):
                for t4 in range(NTT):
                    q_ps = psum_s.tile([128, 512], F32, tag="ps", name="ps")
                    for ct in range(2):
                        nc.tensor.matmul(
                            q_ps[:],
                            qkvT_sb[ct][:, p * 128:(p + 1) * 128],
                            xnq_sb[ct][:, t4 * 512:(t4 + 1) * 512],
                            start=(ct == 0), stop=(ct == 1))
                    nc.scalar.activation(
                        q_sb[p][:, t4 * 512:(t4 + 1) * 512], q_ps[:],
                        mybir.ActivationFunctionType.Identity,
                        bias=qb_sb[p][:])

            # ---------------- vT: [s, head*65] with ones column per head ------
            # vt[:, i*260 + h*65 + j] = V[h*64+j, i*128:...]^T ; col h*65+64 == 1
            vt_sb = data.tile([128, NSC * 260], BF16, tag="vt", name="vt")
            ones_cols = vt_sb[:].rearrange("p (s h c) -> p s h c", s=NSC, c=65)
            nc.vector.memset(ones_cols[:, :, :, 64:65], 1.0)
            for i in range(NSC):
                vt_ps = psum_s.tile([128, C], F32, tag="ps", name="ps")
                for ct in range(2):
                    nc.tensor.matmul(
                        vt_ps[:],
                        xn_sb[ct][:, i * 128:(i + 1) * 128],
                        qkvT_sb[ct][:, 2 * C:3 * C],
                        start=(ct == 0), stop=(ct == 1))
                dst = vt_sb[:, i * 260:(i + 1) * 260].rearrange(
                    "p (h c) -> p h c", c=65)[:, :, 0:64]
                nc.vector.tensor_add(
                    dst,
                    vt_ps[:].rearrange("p (h c) -> p h c", c=64),
                    bvT_sb[:].rearrange("p (h c) -> p h c", c=64))

            # ---------------- attention + proj ----------------
            # pending_proj: ah tile of the previous tt, proj emitted inside the
            # next tt's chunk loop so the PE never waits on the normalize chain.
            pending_proj = [None]

            def emit_proj(tt, ah_pair, oc):
                pr_ps = psum_s.tile([128, 512], F32, tag="ps", name="ps")
                for h in range(HEADS):
                    nc.tensor.matmul(
                        pr_ps[:],
                        projT_sb[h][:, oc * 128:(oc + 1) * 128],
                        ah_pair[h // 2][:, (h % 2) * 512:(h % 2 + 1) * 512],
                        start=(h == 0), stop=(h == HEADS - 1))
                o2 = ao.tile([128, 512], F32, tag="o2", name="o2")
                # out = (proj_psum + proj_bias) + residual, one fused DVE op
                nc.vector.scalar_tensor_tensor(
                    out=o2[:], in0=pr_ps[:], scalar=pb_sb[oc][:],
                    in1=xq_sb[oc][:, tt * 512:(tt + 1) * 512],
                    op0=mybir.AluOpType.add, op1=mybir.AluOpType.add)
                nc.sync.dma_start(
                    out_d.ap()[oc * 128:(oc + 1) * 128, tt * 512:(tt + 1) * 512],
                    o2[:])

            for tt in range(NTT):
                ah_pair = [None, None]
                for p in range(2):

                    def s_matmuls(i, p=p):
                        s_ps = psum_s.tile([128, 1024], F32, tag="ps", name="ps")
                        for u in range(2):
                            nc.tensor.matmul(
                                s_ps[:, u * 512:(u + 1) * 512],
                                k_sb[p][u * 64:(u + 1) * 64, i * 128:(i + 1) * 128],
                                q_sb[p][u * 64:(u + 1) * 64, tt * 512:(tt + 1) * 512],
                                start=True, stop=True,
                                tile_position=(u * 64, 0))
                        return s_ps

                    def emit_exp(i, s_ps):
                        """exp of one score chunk -> bf16 AP for the PV matmul."""
                        if i not in DVE_CHUNKS:
                            p_t = ppool.tile([128, 1024], BF16, tag="pt", name="pt")
                            nc.scalar.activation(p_t[:], s_ps[:],
                                                 mybir.ActivationFunctionType.Exp,
                                                 scale=ACT_SCALE)
                            return p_t[:]
                        p16 = ppool.tile([128, 1024], I16, tag="pt", name="pt")
                        nc.vector.tensor_scalar_add(
                            out=p16[:], in0=s_ps[:], scalar1=BETA)
                        return p16[:].bitcast(BF16)

                    a_ps = psum_a.tile([65, 1024], F32, tag="pa", name="pa")
                    pts = {}
                    for i in range(3):
                        pts[i] = emit_exp(i, s_matmuls(i))
                    for i in range(NSC):
                        p_t = pts.pop(i)
                        for u in range(2):
                            h = 2 * p + u
                            nc.tensor.matmul(
                                a_ps[:, u * 512:(u + 1) * 512],
                                vt_sb[:, i * 260 + h * 65:i * 260 + h * 65 + 65],
                                p_t[:, u * 512:(u + 1) * 512],
                                start=(i == 0), stop=(i == NSC - 1))
                        if i + 3 < NSC:
                            pts[i + 3] = emit_exp(i + 3, s_matmuls(i + 3))
                        # stagger the previous tt's proj into the p=0 loop,
                        # late enough that the gpsimd normalize chain of the
                        # previous iteration has certainly finished (a stalled
                        # proj matmul blocks the in-order PE queue and lets
                        # the HAM clock gate re-throttle)
                        if p == 0 and pending_proj[0] is not None and i in (10, 14):
                            emit_proj(tt - 1, pending_proj[0], 0 if i == 10 else 1)
                            if i == 14:
                                pending_proj[0] = None

                    # evict a_ps to SBUF fast (Act copy) so the single PSUM
                    # accumulator frees for the next (tt, p) iteration; the
                    # normalize chain then runs off the critical path.
                    af = afpool.tile([65, 1024], F32, tag="af", name="af")
                    nc.scalar.copy(af[:], a_ps[:])
                    # custom ops (recip_approx, partition_broadcast) ignore the
                    # AP base partition, so first move the den row to a
                    # partition-0 tile with an ordinary copy.
                    den0 = dn.tile([1, 1024], F32, tag="den0", name="den0")
                    nc.vector.tensor_copy(den0[:], af[64:65, :])
                    d_inv = dn.tile([1, 1024], F32, tag="dinv", name="dinv")
                    nc.vector.reciprocal_approx_fast(d_inv[:], den0[:])
                    d_bc = dn.tile([64, 1024], F32, tag="dbc", name="dbc")
                    nc.gpsimd.partition_broadcast(d_bc[:], d_inv[:])
                    ah = ahpool.tile([64, 1024], BF16, tag="ah", name="ah")
                    nc.gpsimd.tensor_mul(ah[:], af[0:64, :], d_bc[:])
                    ah_pair[p] = ah

                pending_proj[0] = ah_pair

            emit_proj(NTT - 1, pending_proj[0], 0)
            emit_proj(NTT - 1, pending_proj[0], 1)

    nc.compile()
    nc.m = get_hw_module(nc.m)
    return nc


def _host_prep(inputs):
    x = np.asarray(inputs["x"], np.float32)
    gn_w = np.asarray(inputs["gn_weight"], np.float32)
    gn_b = np.asarray(inputs["gn_bias"], np.float32)
    qkv_w = np.asarray(inputs["qkv_w"], np.float32)
    qkv_b = np.asarray(inputs["qkv_b"], np.float32)
    proj_w = np.asarray(inputs["proj_w"], np.float32)
    proj_b = np.asarray(inputs["proj_b"], np.float32)

    W_ = qkv_w * gn_w[None, :]
    b_ = qkv_w @ gn_b + qkv_b
    # fold the Schraudolph ALPHA into the K projection so the score psum
    # arrives pre-scaled (ALPHA * q.k) and the DVE exp is a single add
    W_[C:2 * C, :] *= ALPHA
    b_[C:2 * C] *= ALPHA
    qkvT = np.ascontiguousarray(W_.T).astype(BF)
    # proj_w.T is [c_in(=head*d), c_out]; split head dim so each head's
    # 64 rows sit on partitions 0..63
    projT = np.ascontiguousarray(proj_w.T.reshape(HEADS, D, C)).astype(BF)

    gmat = np.zeros((128, 16), np.float32)
    gmatT = np.zeros((16, 128), np.float32)
    for ch in range(128):
        gmat[ch, ch // GS] = 1.0 / GS
        gmatT[ch // GS, ch] = 1.0
    shared = {
        "qkvT": qkvT,
        "qkvb": b_.reshape(3 * C, 1).astype(np.float32),
        "projT": projT,
        "projb": proj_b.reshape(C, 1).astype(np.float32),
        "gmat": gmat,
        "gmatT": gmatT,
    }
    x3 = x.reshape(B, C, T)
    in_maps = []
    for j in range(8):
        b, hf = j // 2, j % 2
        m = dict(shared)
        m["xb"] = np.ascontiguousarray(x3[b])
        m["xq"] = np.ascontiguousarray(x3[b][:, hf * TQ:(hf + 1) * TQ])
        in_maps.append(m)
    return x, in_maps


def kernel(**inputs) -> np.ndarray:
    if "nc" not in _CACHED:
        _CACHED["nc"] = _build()
    nc = _CACHED["nc"]
    x, in_maps = _host_prep(inputs)
    res = bass_utils.run_bass_kernel_spmd(nc, in_maps, core_ids=list(range(8)))
    out = np.zeros((B, C, T), np.float32)
    for j in range(8):
        b, hf = j // 2, j % 2
        out[b][:, hf * TQ:(hf + 1) * TQ] = np.asarray(
            res.results[j]["out"], np.float32)
    return out.reshape(B, C, Himg, Wimg)


# revision 14
# speedup vs baseline: 1.2931x; 1.0359x over previous
"""AttentionBlock (GroupNorm -> qkv conv1x1 -> 4-head attention -> proj -> residual)
as a distributed Bass/Tile kernel on 8 TRN2 NeuronCores.

Sharding: core j handles batch b = j//2 and query-half h = j%2 (2048 of the 4096
spatial positions). K/V are computed full-length per core (cheap), so the proj
output slices are disjoint across cores and no collectives are needed.

Engine split: exp of the attention scores is the hard bottleneck (only the Act
engine has exp, 128 lanes @ 1.2 GHz). So ~2/3 of the score chunks use a
Schraudolph-style exp approximation on the Vector engine (one tensor_scalar
producing int16 bits that reinterpret as bf16), the rest use exact exp on Act.
Denominator broadcast + normalize run on GPSIMD; GN apply runs on Act.

Self-contained: hardcodes all shapes; host side only reshapes/shards inputs,
transposes/folds weights, and reassembles the 8 output slices.
"""
import numpy as np
import ml_dtypes

import concourse.bass as bass
import concourse.bacc as bacc
import concourse.tile as tile
from concourse import mybir
from concourse import bass_utils
from concourse.bass_interp import get_hw_module

F32 = mybir.dt.float32
BF16 = mybir.dt.bfloat16
I16 = mybir.dt.int16
BF = ml_dtypes.bfloat16

B, C, Himg, Wimg = 4, 256, 64, 64
T = Himg * Wimg            # 4096 tokens
HEADS, D = 4, 64           # 4 heads x 64 dims
GROUPS, GS = 32, 8         # groupnorm: 32 groups of 8 channels
EPS = 1e-5
TQ = T // 2                # queries per core (2048)
NTT = TQ // 512            # query tiles of 512
NSC = T // 128             # key chunks of 128
SCALE = 1.0 / np.sqrt(D)

# Schraudolph exp-approx constants (bf16 bit trick):
#   bits16 = round(ALPHA * s + BETA);  bits16 viewed as bf16 ~ exp(SCALE * s)
# ALPHA is folded into the K weights host-side, so the kernel's score psum
# already holds ALPHA * s and the DVE exp is a single scalar-add.
# BETA tuned so the approximation is unbiased (E[approx/exp] = 1), which makes
# mixing approx (DVE) and exact (Act) chunks inside one softmax safe.
ALPHA = float(128.0 * np.log2(np.e) * SCALE)
BETA = float(127.0 * 128.0 - 7.5)
# exact-exp scale for the Act engine given the pre-scaled scores
ACT_SCALE = float(np.log(2.0) / 128.0)
# chunks of each (tt, p) iteration computed on the DVE (approx); the rest use
# exact exp on Act. Perfect alternation keeps both engine queues shallow, and
# ending on a DVE chunk leaves Act free for the a_ps eviction copy.
DVE_CHUNKS = frozenset(i for i in range(NSC) if i % 2 == 1)

_CACHED = {}


def _build():
    nc = bacc.Bacc("TRN2", target_bir_lowering=False, debug=False,
                   enable_asserts=False, num_devices=8)

    xb_d = nc.dram_tensor("xb", [C, T], F32, kind="ExternalInput")
    xq_d = nc.dram_tensor("xq", [C, TQ], F32, kind="ExternalInput")
    qkvT_d = nc.dram_tensor("qkvT", [C, 3 * C], BF16, kind="ExternalInput")
    qkvb_d = nc.dram_tensor("qkvb", [3 * C, 1], F32, kind="ExternalInput")
    projT_d = nc.dram_tensor("projT", [HEADS, D, C], BF16, kind="ExternalInput")
    projb_d = nc.dram_tensor("projb", [C, 1], F32, kind="ExternalInput")
    gmat_d = nc.dram_tensor("gmat", [128, 16], F32, kind="ExternalInput")
    gmatT_d = nc.dram_tensor("gmatT", [16, 128], F32, kind="ExternalInput")
    out_d = nc.dram_tensor("out", [C, TQ], F32, kind="ExternalOutput")

    with tile.TileContext(nc) as tc:
        with (
            tc.tile_pool(name="consts", bufs=1) as consts,
            tc.tile_pool(name="data", bufs=1) as data,
            tc.tile_pool(name="gn", bufs=1) as gn,
            tc.tile_pool(name="pt", bufs=4) as ppool,
            tc.tile_pool(name="dn", bufs=2) as dn,
            tc.tile_pool(name="af", bufs=2) as afpool,
            tc.tile_pool(name="ao", bufs=4) as ao,
            tc.tile_pool(name="ah", bufs=4) as ahpool,
            tc.tile_pool(name="ps", bufs=3, space="PSUM") as psum_s,
            tc.tile_pool(name="pa", bufs=1, space="PSUM") as psum_a,
        ):
            # ---------------- constant / weight loads ----------------
            qkvT_sb, qb_sb, kb_sb, pb_sb = [], [], [], []
            for ct in range(2):
                w = consts.tile([128, 3 * C], BF16, tag=f"qkvT{ct}", name=f"qkvT{ct}")
                nc.sync.dma_start(w[:], qkvT_d.ap()[ct * 128:(ct + 1) * 128, :])
                qkvT_sb.append(w)
                qb = consts.tile([128, 1], F32, tag=f"qb{ct}", name=f"qb{ct}")
                nc.sync.dma_start(qb[:], qkvb_d.ap()[ct * 128:(ct + 1) * 128, :])
                qb_sb.append(qb)
                kb = consts.tile([128, 1], F32, tag=f"kb{ct}", name=f"kb{ct}")
                nc.sync.dma_start(kb[:], qkvb_d.ap()[C + ct * 128:C + (ct + 1) * 128, :])
                kb_sb.append(kb)
                pb = consts.tile([128, 1], F32, tag=f"pb{ct}", name=f"pb{ct}")
                nc.sync.dma_start(pb[:], projb_d.ap()[ct * 128:(ct + 1) * 128, :])
                pb_sb.append(pb)
            projT_sb = []
            for h in range(HEADS):
                pw = consts.tile([D, C], BF16, tag=f"projT{h}", name=f"projT{h}")
                nc.sync.dma_start(pw[:], projT_d.ap()[h])
                projT_sb.append(pw)
            gmat_sb = consts.tile([128, 16], F32, tag="gmat", name="gmat")
            nc.sync.dma_start(gmat_sb[:], gmat_d.ap()[:])
            gmatT_sb = consts.tile([16, 128], F32, tag="gmatT", name="gmatT")
            nc.sync.dma_start(gmatT_sb[:], gmatT_d.ap()[:])
            # v-bias broadcast along partitions: [128, 256] from qkvb[512:768]
            bvT_sb = consts.tile([128, C], F32, tag="bvT", name="bvT")
            bvT_src = bass.AP(tensor=qkvb_d, offset=2 * C, ap=[[0, 128], [1, C]])
            nc.sync.dma_start(bvT_sb[:], bvT_src)
            eps_t = gn.tile([16, 1], F32, tag="eps", name="eps")
            nc.vector.memset(eps_t[:], EPS)

            # ---------------- x loads (column-split so stats start early) ----
            xb_sb, xq_sb = [], []
            for ct in range(2):
                xt = data.tile([128, T], F32, tag=f"xb{ct}", name=f"xb{ct}")
                for t8 in range(8):
                    nc.sync.dma_start(
                        xt[:, t8 * 512:(t8 + 1) * 512],
                        xb_d.ap()[ct * 128:(ct + 1) * 128, t8 * 512:(t8 + 1) * 512])
                xb_sb.append(xt)
                xqt = data.tile([128, TQ], F32, tag=f"xq{ct}", name=f"xq{ct}")
                for t4 in range(4):
                    nc.sync.dma_start(
                        xqt[:, t4 * 512:(t4 + 1) * 512],
                        xq_d.ap()[ct * 128:(ct + 1) * 128, t4 * 512:(t4 + 1) * 512])
                xq_sb.append(xqt)

            # ---------------- GroupNorm statistics ----------------
            # per-channel mean/var via bn_stats, then 8-channel group
            # aggregation via tiny PE matmuls with the group matrices.
            stats2 = []
            for ct in range(2):
                st = gn.tile([128, 8, 6], F32, tag=f"st{ct}", name=f"st{ct}")
                for i in range(8):
                    nc.vector.bn_stats(st[:, i, :], xb_sb[ct][:, i * 512:(i + 1) * 512])
                mv = gn.tile([128, 2], F32, tag=f"mv{ct}", name=f"mv{ct}")
                nc.vector.bn_aggr(mv[:], st[:])
                s2 = gn.tile([128, 2], F32, tag=f"s2{ct}", name=f"s2{ct}")
                nc.vector.tensor_copy(s2[:, 0:1], mv[:, 0:1])
                m2 = gn.tile([128, 1], F32, tag=f"m2{ct}", name=f"m2{ct}")
                nc.vector.tensor_mul(m2[:], mv[:, 0:1], mv[:, 0:1])
                nc.vector.tensor_add(s2[:, 1:2], m2[:], mv[:, 1:2])
                stats2.append(s2)

            # group (mean, E[x^2]) per channel tile -> [16, 2] each
            gs_ps, bc_sb = [], []
            vg = gn.tile([16, 2], F32, tag="vg", name="vg")
            for ct in range(2):
                g1 = psum_s.tile([16, 2], F32, tag="ps", name="ps")
                nc.tensor.matmul(g1[:], gmat_sb[:], stats2[ct][:],
                                 start=True, stop=True)
                gsb = gn.tile([16, 2], F32, tag=f"gsb{ct}", name=f"gsb{ct}")
                nc.vector.tensor_copy(gsb[:], g1[:])
                gs_ps.append(gsb)
                m2g = gn.tile([16, 1], F32, tag=f"m2g{ct}", name=f"m2g{ct}")
                nc.vector.tensor_mul(m2g[:], gsb[:, 0:1], gsb[:, 0:1])
                nc.vector.tensor_sub(vg[:, ct:ct + 1], gsb[:, 1:2], m2g[:])
            # rsqrt via ln/exp — both live in the same Act table set as the
            # softmax exp, so the kernel needs exactly one ACT_TABLE_LOAD
            # (the Sqrt set would cost two ~1.3us swaps).
            lnv = gn.tile([16, 2], F32, tag="lnv", name="lnv")
            nc.scalar.activation(lnv[:], vg[:], mybir.ActivationFunctionType.Ln,
                                 bias=eps_t[:])
            rg = gn.tile([16, 2], F32, tag="rg", name="rg")
            nc.scalar.activation(rg[:], lnv[:], mybir.ActivationFunctionType.Exp,
                                 scale=-0.5)
            negmr_sb = []
            for ct in range(2):
                bcv = gn.tile([16, 2], F32, tag=f"bcv{ct}", name=f"bcv{ct}")
                nc.vector.tensor_copy(bcv[:, 0:1], gs_ps[ct][:, 0:1])
                nc.vector.tensor_copy(bcv[:, 1:2], rg[:, ct:ct + 1])
                b1 = psum_s.tile([128, 2], F32, tag="ps", name="ps")
                nc.tensor.matmul(b1[:], gmatT_sb[:], bcv[:],
                                 start=True, stop=True)
                bsb = gn.tile([128, 2], F32, tag=f"bc{ct}", name=f"bc{ct}")
                nc.vector.tensor_copy(bsb[:], b1[:])
                bc_sb.append(bsb)
                # -mean * rstd, for the fused (x*r + b) GN apply on Act
                nmr = gn.tile([128, 1], F32, tag=f"nmr{ct}", name=f"nmr{ct}")
                nc.vector.scalar_tensor_tensor(
                    out=nmr[:], in0=bsb[:, 0:1], scalar=-1.0, in1=bsb[:, 1:2],
                    op0=mybir.AluOpType.mult, op1=mybir.AluOpType.mult)
                negmr_sb.append(nmr)

            # ---------------- apply GN -> xn (bf16) -------------------------
            # spread across gpsimd + Act + DVE, ct-interleaved so the first
            # qkv matmuls (which read chunk 0 of BOTH ct tiles) start asap
            xn_sb = [data.tile([128, T], BF16, tag=f"xn{ct}", name=f"xn{ct}")
                     for ct in range(2)]
            xnq_sb = [data.tile([128, TQ], BF16, tag=f"xnq{ct}", name=f"xnq{ct}")
                      for ct in range(2)]
            xn_jobs = []
            for c in range(4):
                for ct in range(2):
                    xn_jobs.append((xn_sb[ct], xb_sb[ct], ct, c))
            for c in range(2):
                for ct in range(2):
                    xn_jobs.append((xnq_sb[ct], xq_sb[ct], ct, c))
            engines = [nc.scalar, nc.gpsimd, nc.vector, nc.scalar,
                       nc.gpsimd, nc.scalar, nc.vector, nc.gpsimd,
                       nc.scalar, nc.gpsimd, nc.vector, nc.scalar]
            for idx, (dst, src, ct, c) in enumerate(xn_jobs):
                eng = engines[idx]
                o = dst[:, c * 1024:(c + 1) * 1024]
                i_ = src[:, c * 1024:(c + 1) * 1024]
                if eng is nc.scalar:
                    nc.scalar.activation(
                        o, i_, mybir.ActivationFunctionType.Identity,
                        scale=bc_sb[ct][:, 1:2], bias=negmr_sb[ct][:])
                else:
                    eng.tensor_scalar(
                        out=o, in0=i_,
                        scalar1=bc_sb[ct][:, 1:2], scalar2=negmr_sb[ct][:],
                        op0=mybir.AluOpType.mult, op1=mybir.AluOpType.add)

            # ---------------- PE warm-up burst ----------------
            # ~4us of throwaway matmuls gated on the first xn chunk, so the
            # HAM clock gate reaches K=8/8 right as the real qkv matmuls
            # start (otherwise the whole qkv phase runs at 1.2 GHz).
            for w in range(18):
                junk_ps = psum_s.tile([128, 512], F32, tag="ps", name="ps")
                nc.tensor.matmul(
                    junk_ps[:], qkvT_sb[0][:, 0:128],
                    xn_sb[0][:, 0:512], start=True, stop=True)

            # ---------------- K (full length), Q (this half) ----------------
            # psum->sbuf copies (+ per-partition bias) run on the Act engine,
            # which is otherwise idle here; DVE keeps the vt adds below.
            k_sb = [data.tile([128, T], BF16, tag=f"k{p}", name=f"k{p}")
                    for p in range(2)]
            for p in range(2):
                for t8 in range(8):
                    kv_ps = psum_s.tile([128, 512], F32, tag="ps", name="ps")
                    for ct in range(2):
                        nc.tensor.matmul(
                            kv_ps[:],
                            qkvT_sb[ct][:, C + p * 128:C + (p + 1) * 128],
                            xn_sb[ct][:, t8 * 512:(t8 + 1) * 512],
                            start=(ct == 0), stop=(ct == 1))
                    nc.scalar.activation(
                        k_sb[p][:, t8 * 512:(t8 + 1) * 512], kv_ps[:],
                        mybir.ActivationFunctionType.Identity,
                        bias=kb_sb[p][:])

            q_sb = [data.tile([128, TQ], BF16, tag=f"q{p}", name=f"q{p}")
                    for p in range(2)]
            for p in range(2):
                for t4 in range(NTT):
                    q_ps = psum_s.tile([128, 512], F32, tag="ps", name="ps")
                    for ct in range(2):
                        nc.tensor.matmul(
                            q_ps[:],
                            qkvT_sb[ct][:, p * 128:(p + 1) * 128],
                            xnq_sb[ct][:, t4 * 512:(t4 + 1) * 512],
                            start=(ct == 0), stop=(ct == 1))
                    nc.scalar.activation(
                        q_sb[p][:, t4 * 512:(t4 + 1) * 512], q_ps[:],
                        mybir.ActivationFunctionType.Identity,
                        bias=qb_sb[p][:])

            # ---------------- vT: [s, head*65] with ones column per head ------
            # vt[:, i*260 + h*65 + j] = V[h*64+j, i*128:...]^T ; col h*65+64 == 1
            vt_sb = data.tile([128, NSC * 260], BF16, tag="vt", name="vt")
            ones_cols = vt_sb[:].rearrange("p (s h c) -> p s h c", s=NSC, c=65)
            nc.vector.memset(ones_cols[:, :, :, 64:65], 1.0)
            for i in range(NSC):
                vt_ps = psum_s.tile([128, C], F32, tag="ps", name="ps")
                for ct in range(2):
                    nc.tensor.matmul(
                        vt_ps[:],
                        xn_sb[ct][:, i * 128:(i + 1) * 128],
                        qkvT_sb[ct][:, 2 * C:3 * C],
                        start=(ct == 0), stop=(ct == 1))
                dst = vt_sb[:, i * 260:(i + 1) * 260].rearrange(
                    "p (h c) -> p h c", c=65)[:, :, 0:64]
                nc.vector.tensor_add(
                    dst,
                    vt_ps[:].rearrange("p (h c) -> p h c", c=64),
                    bvT_sb[:].rearrange("p (h c) -> p h c", c=64))

            # ---------------- attention + proj ----------------
            # pending_proj: ah tile of the previous tt, proj emitted inside the
            # next tt's chunk loop so the PE never waits on the normalize chain.
            pending_proj = [None]

            def emit_proj(tt, ah_pair, oc):
                pr_ps = psum_s.tile([128, 512], F32, tag="ps", name="ps")
                for h in range(HEADS):
                    nc.tensor.matmul(
                        pr_ps[:],
                        projT_sb[h][:, oc * 128:(oc + 1) * 128],
                        ah_pair[h // 2][:, (h % 2) * 512:(h % 2 + 1) * 512],
                        start=(h == 0), stop=(h == HEADS - 1))
                o2 = ao.tile([128, 512], F32, tag="o2", name="o2")
                # out = (proj_psum + proj_bias) + residual, one fused DVE op
                nc.vector.scalar_tensor_tensor(
                    out=o2[:], in0=pr_ps[:], scalar=pb_sb[oc][:],
                    in1=xq_sb[oc][:, tt * 512:(tt + 1) * 512],
                    op0=mybir.AluOpType.add, op1=mybir.AluOpType.add)
                nc.sync.dma_start(
                    out_d.ap()[oc * 128:(oc + 1) * 128, tt * 512:(tt + 1) * 512],
                    o2[:])

            for tt in range(NTT):
                ah_pair = [None, None]
                for p in range(2):

                    def s_matmuls(i, p=p):
                        s_ps = psum_s.tile([128, 1024], F32, tag="ps", name="ps")
                        for u in range(2):
                            nc.tensor.matmul(
                                s_ps[:, u * 512:(u + 1) * 512],
                                k_sb[p][u * 64:(u + 1) * 64, i * 128:(i + 1) * 128],
                                q_sb[p][u * 64:(u + 1) * 64, tt * 512:(tt + 1) * 512],
                                start=True, stop=True,
                                tile_position=(u * 64, 0))
                        return s_ps

                    def emit_exp(i, s_ps):
                        """exp of one score chunk -> bf16 AP for the PV matmul."""
                        if i not in DVE_CHUNKS:
                            p_t = ppool.tile([128, 1024], BF16, tag="pt", name="pt")
                            nc.scalar.activation(p_t[:], s_ps[:],
                                                 mybir.ActivationFunctionType.Exp,
                                                 scale=ACT_SCALE)
                            return p_t[:]
                        p16 = ppool.tile([128, 1024], I16, tag="pt", name="pt")
                        nc.vector.tensor_scalar_add(
                            out=p16[:], in0=s_ps[:], scalar1=BETA)
                        return p16[:].bitcast(BF16)

                    a_ps = psum_a.tile([65, 1024], F32, tag="pa", name="pa")
                    pts = {}
                    for i in range(3):
                        pts[i] = emit_exp(i, s_matmuls(i))
                    for i in range(NSC):
                        p_t = pts.pop(i)
                        for u in range(2):
                            h = 2 * p + u
                            nc.tensor.matmul(
                                a_ps[:, u * 512:(u + 1) * 512],
                                vt_sb[:, i * 260 + h * 65:i * 260 + h * 65 + 65],
                                p_t[:, u * 512:(u + 1) * 512],
                                start=(i == 0), stop=(i == NSC - 1))
                        if i + 3 < NSC:
                            pts[i + 3] = emit_exp(i + 3, s_matmuls(i + 3))
                        # stagger the previous tt's proj into the p=0 loop,
                        # late enough that the gpsimd normalize chain of the
                        # previous iteration has certainly finished (a stalled
                        # proj matmul blocks the in-order PE queue and lets
                        # the HAM clock gate re-throttle)
                        if p == 0 and pending_proj[0] is not None and i in (14, 18):
                            emit_proj(tt - 1, pending_proj[0], 0 if i == 14 else 1)
                            if i == 18:
                                pending_proj[0] = None

                    # evict a_ps to SBUF fast (Act copy) so the single PSUM
                    # accumulator frees for the next (tt, p) iteration; the
                    # normalize chain then runs off the critical path.
                    af = afpool.tile([65, 1024], F32, tag="af", name="af")
                    nc.scalar.copy(af[:], a_ps[:])
                    # custom ops (recip_approx, partition_broadcast) ignore the
                    # AP base partition, so first move the den row to a
                    # partition-0 tile with an ordinary copy.
                    den0 = dn.tile([1, 1024], F32, tag="den0", name="den0")
                    nc.vector.tensor_copy(den0[:], af[64:65, :])
                    d_inv = dn.tile([1, 1024], F32, tag="dinv", name="dinv")
                    nc.vector.reciprocal_approx_fast(d_inv[:], den0[:])
                    d_bc = dn.tile([64, 1024], F32, tag="dbc", name="dbc")
                    nc.gpsimd.partition_broadcast(d_bc[:], d_inv[:])
                    ah = ahpool.tile([64, 1024], BF16, tag="ah", name="ah")
                    nc.gpsimd.tensor_mul(ah[:], af[0:64, :], d_bc[:])
                    ah_pair[p] = ah

                pending_proj[0] = ah_pair

            emit_proj(NTT - 1, pending_proj[0], 0)
            emit_proj(NTT - 1, pending_proj[0], 1)

    nc.compile()
    nc.m = get_hw_module(nc.m)
    return nc


def _host_prep(inputs):
    x = np.asarray(inputs["x"], np.float32)
    gn_w = np.asarray(inputs["gn_weight"], np.float32)
    gn_b = np.asarray(inputs["gn_bias"], np.float32)
    qkv_w = np.asarray(inputs["qkv_w"], np.float32)
    qkv_b = np.asarray(inputs["qkv_b"], np.float32)
    proj_w = np.asarray(inputs["proj_w"], np.float32)
    proj_b = np.asarray(inputs["proj_b"], np.float32)

    W_ = qkv_w * gn_w[None, :]
    b_ = qkv_w @ gn_b + qkv_b
    # fold the Schraudolph ALPHA into the K projection so the score psum
    # arrives pre-scaled (ALPHA * q.k) and the DVE exp is a single add
    W_[C:2 * C, :] *= ALPHA
    b_[C:2 * C] *= ALPHA
    qkvT = np.ascontiguousarray(W_.T).astype(BF)
    # proj_w.T is [c_in(=head*d), c_out]; split head dim so each head's
    # 64 rows sit on partitions 0..63
    projT = np.ascontiguousarray(proj_w.T.reshape(HEADS, D, C)).astype(BF)

    gmat = np.zeros((128, 16), np.float32)
    gmatT = np.zeros((16, 128), np.float32)
    for ch in range(128):
        gmat[ch, ch // GS] = 1.0 / GS
        gmatT[ch // GS, ch] = 1.0
    shared = {
        "qkvT": qkvT,
        "qkvb": b_.reshape(3 * C, 1).astype(np.float32),
        "projT": projT,
        "projb": proj_b.reshape(C, 1).astype(np.float32),
        "gmat": gmat,
        "gmatT": gmatT,
    }
    x3 = x.reshape(B, C, T)
    in_maps = []
    for j in range(8):
        b, hf = j // 2, j % 2
        m = dict(shared)
        m["xb"] = np.ascontiguousarray(x3[b])
        m["xq"] = np.ascontiguousarray(x3[b][:, hf * TQ:(hf + 1) * TQ])
        in_maps.append(m)
    return x, in_maps


def kernel(**inputs) -> np.ndarray:
    if "nc" not in _CACHED:
        _CACHED["nc"] = _build()
    nc = _CACHED["nc"]
    x, in_maps = _host_prep(inputs)
    res = bass_utils.run_bass_kernel_spmd(nc, in_maps, core_ids=list(range(8)))
    out = np.zeros((B, C, T), np.float32)
    for j in range(8):
        b, hf = j // 2, j % 2
        out[b][:, hf * TQ:(hf + 1) * TQ] = np.asarray(
            res.results[j]["out"], np.float32)
    return out.reshape(B, C, Himg, Wimg)
